# revision 1
# baseline (speedup 1.0000x reference)
"""Trainium2 Bass kernel for DragonHGT (heterogeneous graph transformer layer).

Strategy (8 NeuronCores, no collectives):
  - Shard edges by DESTINATION node range: core i owns dst nodes [i*12500, (i+1)*12500)
    of both node types. All segment ops (softmax denom, aggregation) become core-local.
  - Fold per-relation transforms into host-fused weights:
      qr = q @ a_rel^T * scale * p_rel   (folded into Wqr per relation, dst-side)
      vr = v @ m_rel                     (folded into Wvr per relation, src-side)
    so logits = <qr[dst], k[src]> per head and messages need no per-edge small matmuls.
  - Skip segment-max (logits are O(6) here; exp is safe in fp32/fp16 range) and
    normalize AFTER aggregation: agg = (sum_e e_e * vr_src) / (sum_e e_e).
  - Host bucket-sorts edges by (core, superchunk-of-1250-dst, src-subtable-of-25k, dst)
    so that src gathers use int16 dma_gather (fast SWDGE path) and the segment-sum
    is a one-hot matmul into PSUM per 128-node chunk.
  - Tables (k | vr...) are built on-device (replicated across cores) as fp16 HBM
    tables, then gathered per edge with dma_gather (k,qr transposed; vr plain).
"""

import math

import numpy as np

P = 128
NN = 100000          # nodes per type
C = 128
H = 8
DH = 16
M = 8                # cores
NT = NN // M         # 12500 dst rows per core
SCN = 1250           # dst nodes per superchunk
NSC = NT // SCN      # 10 superchunks per core
NCH = 10             # 128-node chunks per superchunk (9*128 + 98)
SUBT = 4             # src subtables
SUBN = NN // SUBT    # 25000
NPAD = 782 * 128     # 100096 (full tables padded)
NTPAD = 98 * 128     # 12544  (dst tables padded)
SCALE = 1.0 / math.sqrt(DH)

# relations: (edge_key, src_type, dst_type)
RELS = [("eAB", 0, 1), ("eBA", 1, 0), ("eAA", 0, 0)]

_CACHE = {}


def _sigmoid(x):
    return 1.0 / (1.0 + np.exp(-x))


def _blockdiag(mats):
    """mats: [H, DH, DH] -> [C, C] block diagonal."""
    out = np.zeros((C, C), np.float32)
    for h in range(H):
        out[h * DH:(h + 1) * DH, h * DH:(h + 1) * DH] = mats[h]
    return out


def _wrap16(arr_i16):
    """[R] int16 -> [128, R//16] wrapped (idx j at [j%16, j//16]) replicated to 128 partitions."""
    R = arr_i16.shape[0]
    w = arr_i16.reshape(R // 16, 16).T  # [16, R/16]
    return np.tile(w, (8, 1))


def _wrap128(arr):
    """[R] -> [128, R//128] (edge j at [j%128, j//128])."""
    R = arr.shape[0]
    return np.ascontiguousarray(arr.reshape(R // 128, 128).T)


def _host_prep(inputs):
    """Returns (meta, per_core_inputs). meta is SPMD-identical; arrays differ per core."""
    xA = np.asarray(inputs["xA"], np.float32)
    xB = np.asarray(inputs["xB"], np.float32)
    Wk = np.asarray(inputs["Wk"], np.float32)
    bk = np.asarray(inputs["bk"], np.float32)
    Wq = np.asarray(inputs["Wq"], np.float32)
    bq = np.asarray(inputs["bq"], np.float32)
    Wv = np.asarray(inputs["Wv"], np.float32)
    bv = np.asarray(inputs["bv"], np.float32)
    Wa = np.asarray(inputs["Wa"], np.float32)
    ba = np.asarray(inputs["ba"], np.float32)
    skip = np.asarray(inputs["skip"], np.float32)
    a_rel = np.asarray(inputs["a_rel"], np.float32)
    m_rel = np.asarray(inputs["m_rel"], np.float32)
    p_rel = np.asarray(inputs["p_rel"], np.float32)

    beta = _sigmoid(skip)  # [2]

    # ---- fused weights ----
    # A-type src table: [ kA | vr(rel0) | vr(rel2) ]  (rel0: A->B, rel2: A->A)
    blkM = [_blockdiag(m_rel[r]) for r in range(3)]
    wfa = np.concatenate([Wk[0], Wv[0] @ blkM[0], Wv[0] @ blkM[2]], axis=1)  # [128,384]
    bfa = np.concatenate([bk[0], bv[0] @ blkM[0], bv[0] @ blkM[2]])          # [384]
    wfb = np.concatenate([Wk[1], Wv[1] @ blkM[1]], axis=1)                   # [128,256]
    bfb = np.concatenate([bk[1], bv[1] @ blkM[1]])
    # qr weights: qr_r = q_t(r) @ blkdiag(a_rel[r].T) * scale * p_rel[r,h]
    blkQ = []
    for r in range(3):
        mats = [a_rel[r, h].T * (SCALE * p_rel[r, h]) for h in range(H)]
        blkQ.append(_blockdiag(np.stack(mats)))
    # dst types: rel0 -> B, rel1 -> A, rel2 -> A
    wqb = Wq[1] @ blkQ[0]
    bqb = bq[1] @ blkQ[0]
    wqa = np.concatenate([Wq[0] @ blkQ[1], Wq[0] @ blkQ[2]], axis=1)  # [128,256]
    bqa = np.concatenate([bq[0] @ blkQ[1], bq[0] @ blkQ[2]])

    # ---- consts ----
    iota = np.tile(np.arange(SCN + 30, dtype=np.float32)[None, :NCH * 128], (P, 1)).astype(np.float16)
    blkd = np.zeros((C, H), np.float16)
    for h in range(H):
        blkd[h * DH:(h + 1) * DH, h] = 1.0
    ones1 = np.ones((1, C), np.float16)
    ident = np.eye(P, dtype=np.float16)

    # ---- per-type padded fp16 x ----
    def pad_rows(a, n):
        out = np.zeros((n, a.shape[1]), a.dtype)
        out[: a.shape[0]] = a
        return out

    xA16 = np.ascontiguousarray(pad_rows(xA.astype(np.float16), NPAD).T)  # [C, NPAD]
    xB16 = np.ascontiguousarray(pad_rows(xB.astype(np.float16), NPAD).T)

    # ---- edge prep ----
    meta = {"Rt": [], "visits": [], "tot16": [], "tot128": []}
    per_core = [dict() for _ in range(M)]
    rng_extra = 0
    for ri, (ekey, styp, dtyp) in enumerate(RELS):
        e = np.asarray(inputs[ekey])
        src = e[0].astype(np.int64)
        dst = e[1].astype(np.int64)
        core = dst // NT
        scid = (dst % NT) // SCN
        sub = src // SUBN
        key = (core * NSC + scid) * SUBT + sub
        order = np.lexsort((dst, key))
        src_s = src[order]
        dst_s = dst[order]
        key_s = key[order]
        counts = np.bincount(key_s, minlength=M * NSC * SUBT).reshape(M, NSC, SUBT)
        Rt = np.maximum(128, ((counts.max(axis=0) + 127) // 128) * 128)  # [NSC, SUBT]
        starts = np.zeros(M * NSC * SUBT + 1, np.int64)
        np.cumsum(counts.reshape(-1), out=starts[1:])

        # per-core arrays + per-batch chunk spans
        tot16 = int(Rt.sum() // 16)
        tot128 = int(Rt.sum() // 128)
        spans = {}  # (sc, sub, b) -> [cmin, cmax] union over cores
        for m in range(M):
            ik = np.zeros(int(Rt.sum()), np.int16)
            iq = np.zeros(int(Rt.sum()), np.int16)
            dr = np.full(int(Rt.sum()), -1.0, np.float32)
            off = 0
            for sc in range(NSC):
                for su in range(SUBT):
                    R = int(Rt[sc, su])
                    k = (m * NSC + sc) * SUBT + su
                    lo, hi = int(starts[k]), int(starts[k + 1])
                    n = hi - lo
                    ik[off:off + n] = (src_s[lo:hi] - su * SUBN).astype(np.int16)
                    iq[off:off + n] = (dst_s[lo:hi] - m * NT).astype(np.int16)
                    dl = (dst_s[lo:hi] - m * NT - sc * SCN).astype(np.int32)
                    dr[off:off + n] = dl.astype(np.float32)
                    for b in range(R // 128):
                        if b * 128 >= n:
                            break
                        c0 = int(dl[b * 128]) // 128
                        c1 = int(dl[min(b * 128 + 127, n - 1)]) // 128
                        kk = (sc, su, b)
                        if kk in spans:
                            spans[kk][0] = min(spans[kk][0], c0)
                            spans[kk][1] = max(spans[kk][1], c1)
                        else:
                            spans[kk] = [c0, c1]
                    off += R
            per_core[m][f"idxk{ri}"] = _wrap16(ik)
            per_core[m][f"idxq{ri}"] = _wrap16(iq)
            per_core[m][f"drel{ri}"] = _wrap128(dr)

        # build visit lists with per-BANK psum group start/stop flags (PSUM zero
        # regions are 2KB = one bank; only one accumulation group per bank, and
        # start zeroes the whole bank). agg-mm of chunk c goes to bank c//4;
        # every s-mm goes to bank 2.
        visits = []  # [sc][sub][b] -> list of (chunk, ag_start, ag_stop, s_start, s_stop)
        for sc in range(NSC):
            order_v = []  # (sub, b, chunk) program order
            for su in range(SUBT):
                for b in range(int(Rt[sc, su]) // 128):
                    sp = spans.get((sc, su, b))
                    if sp is None:
                        continue
                    for c in range(sp[0], sp[1] + 1):
                        order_v.append((su, b, c))
            seen = set(c for _, _, c in order_v)
            last_su = SUBT - 1
            last_b = int(Rt[sc, last_su]) // 128 - 1
            for c in range(NCH):
                if c not in seen:
                    order_v.append((last_su, last_b, c))
            # matmul program order: per visit, agg-mm then s-mm
            mm_banks = []
            for (su, b, c) in order_v:
                mm_banks.append(c // 4)
                mm_banks.append(2)
            first = {}
            last = {}
            for j, bk in enumerate(mm_banks):
                if bk not in first:
                    first[bk] = j
                last[bk] = j
            vl = [[[] for _ in range(int(Rt[sc, su]) // 128)] for su in range(SUBT)]
            for i, (su, b, c) in enumerate(order_v):
                ja, js = 2 * i, 2 * i + 1
                bka = c // 4
                vl[su][b].append((c, ja == first[bka], ja == last[bka],
                                  js == first[2], js == last[2]))
            visits.append(vl)

        meta["Rt"].append([[int(x) for x in row] for row in Rt])
        meta["visits"].append(visits)
        meta["tot16"].append(tot16)
        meta["tot128"].append(tot128)
        rng_extra += int(Rt.sum())

    meta["beta"] = [float(beta[0]), float(beta[1])]

    # ---- shared (replicated) inputs ----
    shared = {
        "xA16": xA16, "xB16": xB16,
        "wfa": wfa.astype(np.float16), "bfa": bfa.astype(np.float16)[None, :],
        "wfb": wfb.astype(np.float16), "bfb": bfb.astype(np.float16)[None, :],
        "wqa": wqa.astype(np.float16), "bqa": bqa.astype(np.float16)[None, :],
        "wqb": wqb.astype(np.float16), "bqb": bqb.astype(np.float16)[None, :],
        "waa": Wa[0].astype(np.float16), "baa": ba[0].astype(np.float16)[None, :],
        "wab": Wa[1].astype(np.float16), "bab": ba[1].astype(np.float16)[None, :],
        "iota": iota, "blkd": blkd, "ones1": ones1, "ident": ident,
    }
    for m in range(M):
        r0, r1 = m * NT, (m + 1) * NT
        per_core[m]["xad16"] = np.ascontiguousarray(pad_rows(xA[r0:r1].astype(np.float16), NTPAD).T)
        per_core[m]["xbd16"] = np.ascontiguousarray(pad_rows(xB[r0:r1].astype(np.float16), NTPAD).T)
        per_core[m]["xsa"] = np.ascontiguousarray((1.0 - beta[0]) * xA[r0:r1])
        per_core[m]["xsb"] = np.ascontiguousarray((1.0 - beta[1]) * xB[r0:r1])
        per_core[m].update(shared)
    return meta, per_core


def _build_nc(meta):
    import concourse.bacc as bacc
    import concourse.mybir as mybir
    import concourse.tile as tile

    f16 = mybir.dt.float16
    f32 = mybir.dt.float32
    i16 = mybir.dt.int16
    AF = mybir.ActivationFunctionType
    ALU = mybir.AluOpType

    nc = bacc.Bacc("TRN2", target_bir_lowering=False, debug=False, num_swdge_queues=4)

    # ---- I/O ----
    def din(name, shape, dt):
        return nc.dram_tensor(name, shape, dt, kind="ExternalInput")

    xA16 = din("xA16", [C, NPAD], f16)
    xB16 = din("xB16", [C, NPAD], f16)
    xad16 = din("xad16", [C, NTPAD], f16)
    xbd16 = din("xbd16", [C, NTPAD], f16)
    xsa = din("xsa", [NT, C], f32)
    xsb = din("xsb", [NT, C], f32)
    wfa = din("wfa", [C, 384], f16)
    bfa = din("bfa", [1, 384], f16)
    wfb = din("wfb", [C, 256], f16)
    bfb = din("bfb", [1, 256], f16)
    wqa = din("wqa", [C, 256], f16)
    bqa = din("bqa", [1, 256], f16)
    wqb = din("wqb", [C, 128], f16)
    bqb = din("bqb", [1, 128], f16)
    waa = din("waa", [C, C], f16)
    baa = din("baa", [1, C], f16)
    wab = din("wab", [C, C], f16)
    bab = din("bab", [1, C], f16)
    iota_d = din("iota", [P, NCH * 128], f16)
    blkd_d = din("blkd", [C, H], f16)
    ones1_d = din("ones1", [1, C], f16)
    ident_d = din("ident", [P, P], f16)
    idx_d = []
    for r in range(3):
        idx_d.append((
            din(f"idxk{r}", [P, meta["tot16"][r]], i16),
            din(f"idxq{r}", [P, meta["tot16"][r]], i16),
            din(f"drel{r}", [P, meta["tot128"][r]], f32),
        ))
    outA = nc.dram_tensor("outA", [NT, C], f32, kind="ExternalOutput")
    outB = nc.dram_tensor("outB", [NT, C], f32, kind="ExternalOutput")

    Rt = meta["Rt"]
    visits = meta["visits"]
    betaA, betaB = meta["beta"]

    with tile.TileContext(nc) as tc:
        with tc.tile_pool(name="dram", bufs=1, space="DRAM") as dram:
            fusedA = dram.tile([NPAD, 384], f16)
            fusedB = dram.tile([NPAD, 256], f16)
            qra = dram.tile([NTPAD, 256], f16)
            qrb = dram.tile([NTPAD, 128], f16)

            with tc.tile_pool(name="const", bufs=1) as cp:
                iota_sb = cp.tile([P, NCH * 128], f16)
                nc.sync.dma_start(iota_sb[:], iota_d[:])
                blkd_sb = cp.tile([C, H], f16)
                nc.sync.dma_start(blkd_sb[:], blkd_d[:])
                ones1_sb = cp.tile([1, C], f16)
                nc.sync.dma_start(ones1_sb[:], ones1_d[:])
                ident_sb = cp.tile([P, P], f16)
                nc.sync.dma_start(ident_sb[:], ident_d[:])
                w_sb = {}
                for nm, dt_, sh in [("wfa", f16, [C, 384]), ("bfa", f16, [1, 384]),
                                    ("wfb", f16, [C, 256]), ("bfb", f16, [1, 256]),
                                    ("wqa", f16, [C, 256]), ("bqa", f16, [1, 256]),
                                    ("wqb", f16, [C, 128]), ("bqb", f16, [1, 128]),
                                    ("waa", f16, [C, C]), ("baa", f16, [1, C]),
                                    ("wab", f16, [C, C]), ("bab", f16, [1, C])]:
                    t = cp.tile(sh, dt_, tag=nm)
                    nc.sync.dma_start(t[:], {"wfa": wfa, "bfa": bfa, "wfb": wfb, "bfb": bfb,
                                             "wqa": wqa, "bqa": bqa, "wqb": wqb, "bqb": bqb,
                                             "waa": waa, "baa": baa, "wab": wab, "bab": bab}[nm][:])
                    w_sb[nm] = t

                import os as _osr
                _REP = int(_osr.environ.get("KERNEL_REPEAT", "1"))
                for _rep in range(_REP):
                    # ================= PHASE 1: build tables =================
                    with tc.tile_pool(name="prep", bufs=4) as pp, \
                         tc.tile_pool(name="prep_ps", bufs=3, space="PSUM") as pps:

                        def build_table(xdram, nrows, w, b, tbl, ncols):
                            GRP = 4  # node-tiles per DMA batch
                            ntiles = nrows // 128
                            base = 0
                            ii = 0
                            while base < ntiles:
                                grp = min(GRP, ntiles - base)
                                r0 = base * 128
                                xT = pp.tile([P, GRP, P], f16, tag="xT")
                                nc.sync.dma_start(
                                    xT[:, :grp, :], xdram[:, r0:r0 + grp * 128]
                                    .rearrange("c (t n) -> c t n", t=grp))
                                sb = pp.tile([P, GRP, ncols], f16, tag=f"sb{ncols}")
                                for t in range(grp):
                                    ps = pps.tile([P, 384], f32, tag="ps")
                                    nc.tensor.matmul(ps[:, :ncols], xT[:, t, :], w[:],
                                                     start=True, stop=False)
                                    nc.tensor.matmul(ps[:, :ncols], ones1_sb[:], b[:],
                                                     start=False, stop=True)
                                    if ii % 2 == 0:
                                        nc.scalar.copy(sb[:, t, :], ps[:, :ncols])
                                    else:
                                        nc.vector.tensor_copy(sb[:, t, :], ps[:, :ncols])
                                    ii += 1
                                nc.scalar.dma_start(
                                    tbl[r0:r0 + grp * 128, :]
                                    .rearrange("(t n) c -> n t c", n=128), sb[:, :grp, :])
                                base += grp

                        import os as _os1
                        if not _os1.environ.get("KERNEL_NOPREP"):
                            build_table(xA16, NPAD, w_sb["wfa"], w_sb["bfa"], fusedA, 384)
                            build_table(xB16, NPAD, w_sb["wfb"], w_sb["bfb"], fusedB, 256)
                            build_table(xad16, NTPAD, w_sb["wqa"], w_sb["bqa"], qra, 256)
                            build_table(xbd16, NTPAD, w_sb["wqb"], w_sb["bqb"], qrb, 128)

                    # ================= PHASE 2: streaming =================
                    with tc.tile_pool(name="agg", bufs=1) as apool:
                        agg = apool.tile([P, NSC * NCH, C], f32)

                        def out_stage(t):
                            import os as _os2
                            if _os2.environ.get("KERNEL_NOOUT"):
                                return
                            xs_d = xsa if t == 0 else xsb
                            out_d = outA if t == 0 else outB
                            wa = w_sb["waa" if t == 0 else "wab"]
                            bb = w_sb["baa" if t == 0 else "bab"]
                            bt = betaA if t == 0 else betaB
                            with tc.tile_pool(name="op", bufs=4) as op, \
                                 tc.tile_pool(name="ops", bufs=2, space="PSUM") as ops:
                                for slot in range(NSC * NCH):
                                    sc, ch = divmod(slot, NCH)
                                    rows = 98 if ch == 9 else 128
                                    base = sc * SCN + ch * 128
                                    g16 = op.tile([P, C], f16, tag="g16")
                                    nc.scalar.activation(g16[:], agg[:, slot, :], AF.Gelu)
                                    gt = ops.tile([P, C], f16, tag="gt")
                                    nc.tensor.transpose(gt[:], g16[:], ident_sb[:])
                                    gts = op.tile([P, C], f16, tag="gts")
                                    nc.vector.tensor_copy(gts[:], gt[:])
                                    o_ps = ops.tile([P, C], f32, tag="o")
                                    nc.tensor.matmul(o_ps[:], gts[:], wa[:], start=True, stop=False)
                                    nc.tensor.matmul(o_ps[:], ones1_sb[:], bb[:], start=False, stop=True)
                                    xs = op.tile([P, C], f32, tag="xs")
                                    nc.sync.dma_start(xs[:rows, :], xs_d[base:base + rows, :])
                                    ob = op.tile([P, C], f32, tag="ob")
                                    nc.scalar.activation(ob[:], o_ps[:], AF.Copy, scale=float(bt))
                                    res = op.tile([P, C], f32, tag="res")
                                    nc.vector.tensor_add(res[:rows, :], ob[:rows, :], xs[:rows, :])
                                    nc.sync.dma_start(out_d[base:base + rows, :], res[:rows, :])

                        with tc.tile_pool(name="gidx", bufs=1) as gi, \
                             tc.tile_pool(name="gp", bufs=2) as gp, \
                             tc.tile_pool(name="ep", bufs=4) as ep:
                            import os as _os
                            n_rel = int(_os.environ.get("KERNEL_NREL", "3"))
                            for r, (ekey, styp, dtyp) in enumerate(RELS[:n_rel]):
                                ftab, fw = (fusedA, 384) if styp == 0 else (fusedB, 256)
                                vcol = 256 if r == 2 else 128
                                if r == 0:
                                    qtab, qw, qoff = qrb, 128, 0
                                elif r == 1:
                                    qtab, qw, qoff = qra, 256, 0
                                else:
                                    qtab, qw, qoff = qra, 256, 128
                                qap = qtab[:, qoff:qoff + 128]

                                idxk_sb = gi.tile([P, meta["tot16"][r]], i16, tag="idxk")
                                nc.sync.dma_start(idxk_sb[:], idx_d[r][0][:])
                                idxq_sb = gi.tile([P, meta["tot16"][r]], i16, tag="idxq")
                                nc.sync.dma_start(idxq_sb[:], idx_d[r][1][:])
                                drel_sb = gi.tile([P, meta["tot128"][r]], f32, tag="drel")
                                nc.sync.dma_start(drel_sb[:], idx_d[r][2][:])

                                with tc.tile_pool(name=f"agps{r}", bufs=2, space="PSUM") as agps, \
                                     tc.tile_pool(name=f"lps{r}", bufs=2, space="PSUM") as lps:
                                    off16 = 0
                                    off128 = 0
                                    for sc in range(NSC):
                                        ag = agps.tile([P, 3, 512], f32, tag="aggps")
                                        for su in range(SUBT):
                                            R = Rt[r][sc][su]
                                            B = R // 128
                                            kap = ftab[su * SUBN:(su + 1) * SUBN, 0:128]
                                            vap = ftab[su * SUBN:(su + 1) * SUBN, vcol:vcol + 128]
                                            GC = 896  # per-gather idx cap (desc carveout is 1024)
                                            kT = gp.tile([P, 1, R], f16, tag="kT")
                                            for j0 in range(0, R, GC):
                                                n = min(GC, R - j0)
                                                nc.gpsimd.dma_gather(
                                                    kT[:, :, j0:j0 + n], kap,
                                                    idxk_sb[:, off16 + j0 // 16:off16 + (j0 + n) // 16],
                                                    n, n, 128, elem_step=fw, transpose=True, queue_num=0)
                                            qT = gp.tile([P, 1, R], f16, tag="qT")
                                            for j0 in range(0, R, GC):
                                                n = min(GC, R - j0)
                                                nc.gpsimd.dma_gather(
                                                    qT[:, :, j0:j0 + n], qap,
                                                    idxq_sb[:, off16 + j0 // 16:off16 + (j0 + n) // 16],
                                                    n, n, 128, elem_step=qw, transpose=True, queue_num=1)
                                            vr = gp.tile([P, B, 128], f16, tag="vr")
                                            for j0 in range(0, R, GC):
                                                n = min(GC, R - j0)
                                                nc.gpsimd.dma_gather(
                                                    vr[:, j0 // 128:(j0 + n) // 128, :], vap,
                                                    idxk_sb[:, off16 + j0 // 16:off16 + (j0 + n) // 16],
                                                    n, n, 128, elem_step=fw, transpose=False, queue_num=2)
                                            prod = gp.tile([P, R], f16, tag="prod")
                                            if not _os.environ.get("KERNEL_NOPROD"):
                                                nc.vector.tensor_mul(prod[:], kT[:, 0, :], qT[:, 0, :])
                                            else:
                                                prod = kT[:, 0, :].tensor if False else kT
                                                prod = None
                                            prod_ap = (prod[:] if prod is not None else kT[:, 0, :])
                                            e8 = gp.tile([P, B, H], f16, tag="e8")
                                            lpr = lps.tile([P, B, H], f32, tag="lp")
                                            for b in range(B):
                                                nc.tensor.matmul(lpr[:, b, :], prod_ap[:, b * 128:(b + 1) * 128],
                                                                 blkd_sb[:], start=(b == 0), stop=(b == B - 1))
                                            nc.scalar.activation(e8[:], lpr[:], AF.Exp)
                                            msg = gp.tile([P, B, C], f16, tag="msg")
                                            if not _os.environ.get("KERNEL_NOMSGMUL"):
                                                nc.vector.tensor_tensor(
                                                    out=msg[:].rearrange("p b (h d) -> p b h d", d=DH),
                                                    in0=vr[:].rearrange("p b (h d) -> p b h d", d=DH),
                                                    in1=e8[:].to_broadcast([P, B, H, DH]),
                                                    op=ALU.mult)
                                            else:
                                                msg = vr
                                            for b in range(B):
                                                for (ch, ast, asp, sst, ssp) in visits[r][sc][su][b]:
                                                    if not _os.environ.get("KERNEL_NOONEHOT"):
                                                        oh = gp.tile([P, P], f16, tag="oh")
                                                        nc.vector.tensor_scalar(
                                                            oh[:], iota_sb[:, ch * 128:(ch + 1) * 128],
                                                            drel_sb[:, off128 + b:off128 + b + 1],
                                                            None, op0=ALU.is_equal)
                                                        oh_ap = oh[:]
                                                    else:
                                                        oh_ap = iota_sb[:, ch * 128:(ch + 1) * 128]
                                                    bk_, col = divmod(ch, 4)
                                                    nc.tensor.matmul(
                                                        ag[:, bk_, col * 128:col * 128 + 128],
                                                        oh_ap, msg[:, b, :], start=ast, stop=asp)
                                                    nc.tensor.matmul(
                                                        ag[:, 2, 256 + ch * 8:256 + ch * 8 + 8],
                                                        oh_ap, e8[:, b, :], start=sst, stop=ssp)
                                            off16 += R // 16
                                            off128 += B
                                        # epilogue for this superchunk
                                        for ch in range(NCH):
                                            bk_, col = divmod(ch, 4)
                                            a_ap = ag[:, bk_, col * 128:col * 128 + 128]
                                            s_ap = ag[:, 2, 256 + ch * 8:256 + ch * 8 + 8]
                                            rec = ep.tile([P, H], f32, tag="rec")
                                            nc.vector.tensor_scalar(rec[:], s_ap, 1e-16, None, op0=ALU.add)
                                            rec2 = ep.tile([P, H], f32, tag="rec2")
                                            nc.vector.reciprocal(rec2[:], rec[:])
                                            slot = sc * NCH + ch
                                            tgt = agg[:, slot, :].rearrange("p (h d) -> p h d", d=DH)
                                            src_v = a_ap.rearrange("p (h d) -> p h d", d=DH)
                                            if r == 2:
                                                tmp = ep.tile([P, C], f32, tag="tmp")
                                                nc.vector.tensor_tensor(
                                                    out=tmp[:].rearrange("p (h d) -> p h d", d=DH),
                                                    in0=src_v, in1=rec2[:].to_broadcast([P, H, DH]),
                                                    op=ALU.mult)
                                                nc.vector.tensor_add(agg[:, slot, :], agg[:, slot, :], tmp[:])
                                            else:
                                                nc.vector.tensor_tensor(
                                                    out=tgt, in0=src_v,
                                                    in1=rec2[:].to_broadcast([P, H, DH]),
                                                    op=ALU.mult)
                                if r == 0:
                                    out_stage(1)
                            out_stage(0)
    nc.compile()
    return nc


def _meta_key(meta):
    import json
    return json.dumps(meta, sort_keys=True)


def kernel(**inputs):
    meta, per_core = _host_prep(inputs)
    key = _meta_key(meta)
    if key not in _CACHE:
        _CACHE.clear()
        _CACHE[key] = _build_nc(meta)
    nc = _CACHE[key]

    from concourse.bass_utils import run_bass_kernel_spmd
    import os
    trace = bool(int(os.environ.get("KERNEL_TRACE", "0")))
    res = run_bass_kernel_spmd(nc, per_core, core_ids=list(range(M)), trace=trace)
    if trace:
        kernel.last_exec_time_ns = res.exec_time_ns
        kernel.last_trace = res.instructions_and_trace
    outs = res.results
    outA = np.concatenate([outs[m]["outA"] for m in range(M)], axis=0)
    outB = np.concatenate([outs[m]["outB"] for m in range(M)], axis=0)
    return np.stack([outA, outB]).astype(np.float32)



# revision 22
# speedup vs baseline: 1.2789x; 1.2789x over previous
"""Trainium2 Bass kernel for DragonHGT (heterogeneous graph transformer layer).

Strategy (8 NeuronCores, no collectives):
  - Shard edges by DESTINATION node range: core i owns dst nodes [i*12500, (i+1)*12500)
    of both node types. All segment ops (softmax denom, aggregation) become core-local.
  - Fold per-relation transforms into host-fused weights:
      qr = q @ a_rel^T * scale * p_rel   (folded into Wqr per relation, dst-side)
      vr = v @ m_rel                     (folded into Wvr per relation, src-side)
    so logits = <qr[dst], k[src]> per head and messages need no per-edge small matmuls.
  - Skip segment-max (logits are O(6) here; exp is safe in fp32/fp16 range) and
    normalize AFTER aggregation: agg = (sum_e e_e * vr_src) / (sum_e e_e).
  - Host bucket-sorts edges by (core, superchunk-of-1250-dst, src-subtable-of-25k, dst)
    so that src gathers use int16 dma_gather (fast SWDGE path) and the segment-sum
    is a one-hot matmul into PSUM per 128-node chunk.
  - Tables (k | vr...) are built on-device (replicated across cores) as fp16 HBM
    tables, then gathered per edge with dma_gather (k,qr transposed; vr plain).
"""

import math

import numpy as np

P = 128
NN = 100000          # nodes per type
C = 128
H = 8
DH = 16
M = 8                # cores
NT = NN // M         # 12500 dst rows per core
SCN = 1250           # dst nodes per superchunk
NSC = NT // SCN      # 10 superchunks per core
NCH = 10             # 128-node chunks per superchunk (9*128 + 98)
SUBT = 4             # src subtables
SUBN = NN // SUBT    # 25000
NPAD = 782 * 128     # 100096 (full tables padded)
NTPAD = 98 * 128     # 12544  (dst tables padded)
SCALE = 1.0 / math.sqrt(DH)

# relations: (edge_key, src_type, dst_type)
RELS = [("eAB", 0, 1), ("eBA", 1, 0), ("eAA", 0, 0)]

_CACHE = {}


def _sigmoid(x):
    return 1.0 / (1.0 + np.exp(-x))


def _blockdiag(mats):
    """mats: [H, DH, DH] -> [C, C] block diagonal."""
    out = np.zeros((C, C), np.float32)
    for h in range(H):
        out[h * DH:(h + 1) * DH, h * DH:(h + 1) * DH] = mats[h]
    return out


def _wrap16(arr_i16):
    """[R] int16 -> [128, R//16] wrapped (idx j at [j%16, j//16]) replicated to 128 partitions."""
    R = arr_i16.shape[0]
    w = arr_i16.reshape(R // 16, 16).T  # [16, R/16]
    return np.tile(w, (8, 1))


def _wrap128(arr):
    """[R] -> [128, R//128] (edge j at [j%128, j//128])."""
    R = arr.shape[0]
    return np.ascontiguousarray(arr.reshape(R // 128, 128).T)


def _host_prep(inputs):
    """Returns (meta, per_core_inputs). meta is SPMD-identical; arrays differ per core."""
    xA = np.asarray(inputs["xA"], np.float32)
    xB = np.asarray(inputs["xB"], np.float32)
    Wk = np.asarray(inputs["Wk"], np.float32)
    bk = np.asarray(inputs["bk"], np.float32)
    Wq = np.asarray(inputs["Wq"], np.float32)
    bq = np.asarray(inputs["bq"], np.float32)
    Wv = np.asarray(inputs["Wv"], np.float32)
    bv = np.asarray(inputs["bv"], np.float32)
    Wa = np.asarray(inputs["Wa"], np.float32)
    ba = np.asarray(inputs["ba"], np.float32)
    skip = np.asarray(inputs["skip"], np.float32)
    a_rel = np.asarray(inputs["a_rel"], np.float32)
    m_rel = np.asarray(inputs["m_rel"], np.float32)
    p_rel = np.asarray(inputs["p_rel"], np.float32)

    beta = _sigmoid(skip)  # [2]

    # ---- fused weights ----
    # On-the-fly derivation: per edge gather x[src] once; k = x @ Wk (bk dropped
    # exactly: a per-(dst,head) logit shift cancels in the softmax ratio),
    # vr = x @ (Wv @ blkM[r]) + bv @ blkM[r] (bias added via ones-outer matmul).
    blkM = [_blockdiag(m_rel[r]) for r in range(3)]
    wka, wkb = Wk[0], Wk[1]
    wvr = [Wv[(0, 1, 0)[r]] @ blkM[r] for r in range(3)]
    bvr = [bv[(0, 1, 0)[r]] @ blkM[r] for r in range(3)]
    # qr weights: qr_r = q_t(r) @ blkdiag(a_rel[r].T) * scale * p_rel[r,h]
    blkQ = []
    for r in range(3):
        mats = [a_rel[r, h].T * (SCALE * p_rel[r, h]) for h in range(H)]
        blkQ.append(_blockdiag(np.stack(mats)))
    # dst types: rel0 -> B, rel1 -> A, rel2 -> A
    wqb = Wq[1] @ blkQ[0]
    bqb = bq[1] @ blkQ[0]
    wqa = np.concatenate([Wq[0] @ blkQ[1], Wq[0] @ blkQ[2]], axis=1)  # [128,256]
    bqa = np.concatenate([bq[0] @ blkQ[1], bq[0] @ blkQ[2]])

    # ---- consts ----
    iota = np.tile(np.arange(SCN + 30, dtype=np.float32)[None, :NCH * 128], (P, 1)).astype(np.float16)
    blkd = np.zeros((C, H), np.float16)
    for h in range(H):
        blkd[h * DH:(h + 1) * DH, h] = 1.0
    ones1 = np.ones((1, C), np.float16)
    ident = np.eye(P, dtype=np.float16)

    # ---- per-type padded fp16 x ----
    def pad_rows(a, n):
        out = np.zeros((n, a.shape[1]), a.dtype)
        out[: a.shape[0]] = a
        return out

    xArow = pad_rows(xA.astype(np.float16), NPAD)  # [NPAD, C] row-major gather table
    xBrow = pad_rows(xB.astype(np.float16), NPAD)

    # ---- edge prep ----
    meta = {"Rt": [], "visits": [], "tot16": [], "tot128": []}
    per_core = [dict() for _ in range(M)]
    rng_extra = 0
    for ri, (ekey, styp, dtyp) in enumerate(RELS):
        e = np.asarray(inputs[ekey])
        src = e[0].astype(np.int64)
        dst = e[1].astype(np.int64)
        core = dst // NT
        scid = (dst % NT) // SCN
        sub = src // SUBN
        key = (core * NSC + scid) * SUBT + sub
        order = np.lexsort((dst, key))
        src_s = src[order]
        dst_s = dst[order]
        key_s = key[order]
        counts = np.bincount(key_s, minlength=M * NSC * SUBT).reshape(M, NSC, SUBT)
        Rt = np.maximum(128, ((counts.max(axis=0) + 127) // 128) * 128)  # [NSC, SUBT]
        starts = np.zeros(M * NSC * SUBT + 1, np.int64)
        np.cumsum(counts.reshape(-1), out=starts[1:])

        # per-core arrays + per-batch chunk spans
        tot16 = int(Rt.sum() // 16)
        tot128 = int(Rt.sum() // 128)
        spans = {}  # (sc, sub, b) -> [cmin, cmax] union over cores
        for m in range(M):
            ik = np.zeros(int(Rt.sum()), np.int16)
            iq = np.zeros(int(Rt.sum()), np.int16)
            dr = np.full(int(Rt.sum()), -1.0, np.float16)
            off = 0
            for sc in range(NSC):
                for su in range(SUBT):
                    R = int(Rt[sc, su])
                    k = (m * NSC + sc) * SUBT + su
                    lo, hi = int(starts[k]), int(starts[k + 1])
                    n = hi - lo
                    ik[off:off + n] = (src_s[lo:hi] - su * SUBN).astype(np.int16)
                    iq[off:off + n] = (dst_s[lo:hi] - m * NT).astype(np.int16)
                    dl = (dst_s[lo:hi] - m * NT - sc * SCN).astype(np.int32)
                    dr[off:off + n] = dl.astype(np.float16)
                    for b in range(R // 128):
                        if b * 128 >= n:
                            break
                        c0 = int(dl[b * 128]) // 128
                        c1 = int(dl[min(b * 128 + 127, n - 1)]) // 128
                        kk = (sc, su, b)
                        if kk in spans:
                            spans[kk][0] = min(spans[kk][0], c0)
                            spans[kk][1] = max(spans[kk][1], c1)
                        else:
                            spans[kk] = [c0, c1]
                    off += R
            per_core[m][f"idxk{ri}"] = _wrap16(ik)
            per_core[m][f"idxq{ri}"] = _wrap16(iq)
            per_core[m][f"drel{ri}"] = _wrap128(dr)
            per_core[m][f"drln{ri}"] = _wrap128((-dr).astype(np.float32))

        # build visit lists with per-BANK psum group start/stop flags (PSUM zero
        # regions are 2KB = one bank; only one accumulation group per bank, and
        # start zeroes the whole bank). One fused [msg|e8] matmul per visit:
        # chunk c -> bank c//3, offset (c%3)*136 (128 msg cols + 8 exp-sum cols).
        visits = []  # [sc][sub][b] -> list of (chunk, start, stop)
        for sc in range(NSC):
            order_v = []  # (sub, b, chunk) program order
            for su in range(SUBT):
                for b in range(int(Rt[sc, su]) // 128):
                    sp = spans.get((sc, su, b))
                    if sp is None:
                        continue
                    for c in range(sp[0], sp[1] + 1):
                        order_v.append((su, b, c))
            seen_banks = set(c // 3 for _, _, c in order_v)
            last_su = SUBT - 1
            last_b = int(Rt[sc, last_su]) // 128 - 1
            for bk in range(4):
                if bk not in seen_banks:
                    order_v.append((last_su, last_b, bk * 3))
            first = {}
            last = {}
            for j, (_, _, c) in enumerate(order_v):
                bk = c // 3
                if bk not in first:
                    first[bk] = j
                last[bk] = j
            vl = [[[] for _ in range(int(Rt[sc, su]) // 128)] for su in range(SUBT)]
            for j, (su, b, c) in enumerate(order_v):
                bk = c // 3
                vl[su][b].append((c, j == first[bk], j == last[bk]))
            visits.append(vl)

        meta["Rt"].append([[int(x) for x in row] for row in Rt])
        meta["visits"].append(visits)
        meta["tot16"].append(tot16)
        meta["tot128"].append(tot128)
        rng_extra += int(Rt.sum())

    meta["beta"] = [float(beta[0]), float(beta[1])]

    # ---- shared (replicated) inputs ----
    shared = {
        "xArow": xArow, "xBrow": xBrow,
        "wka": wka.astype(np.float16), "wkb": wkb.astype(np.float16),
        "wv0": wvr[0].astype(np.float16), "wv1": wvr[1].astype(np.float16),
        "wv2": wvr[2].astype(np.float16),
        "bv0": bvr[0].astype(np.float16)[None, :], "bv1": bvr[1].astype(np.float16)[None, :],
        "bv2": bvr[2].astype(np.float16)[None, :],
        "wqa": wqa.astype(np.float16), "bqa": bqa.astype(np.float16)[None, :],
        "wqb": wqb.astype(np.float16), "bqb": bqb.astype(np.float16)[None, :],
        "waa": Wa[0].astype(np.float16), "baa": ba[0].astype(np.float16)[None, :],
        "wab": Wa[1].astype(np.float16), "bab": ba[1].astype(np.float16)[None, :],
        "iota": iota, "blkd": blkd, "ones1": ones1, "ident": ident,
    }
    for m in range(M):
        r0, r1 = m * NT, (m + 1) * NT
        per_core[m]["xad16"] = np.ascontiguousarray(pad_rows(xA[r0:r1].astype(np.float16), NTPAD).T)
        per_core[m]["xbd16"] = np.ascontiguousarray(pad_rows(xB[r0:r1].astype(np.float16), NTPAD).T)
        per_core[m]["xsa"] = np.ascontiguousarray((1.0 - beta[0]) * xA[r0:r1])
        per_core[m]["xsb"] = np.ascontiguousarray((1.0 - beta[1]) * xB[r0:r1])
        per_core[m].update(shared)
    return meta, per_core


def _build_nc(meta):
    import concourse.bacc as bacc
    import concourse.mybir as mybir
    import concourse.tile as tile

    f16 = mybir.dt.float16
    f32 = mybir.dt.float32
    i16 = mybir.dt.int16
    AF = mybir.ActivationFunctionType
    ALU = mybir.AluOpType

    nc = bacc.Bacc("TRN2", target_bir_lowering=False, debug=False, num_swdge_queues=4)

    # ---- I/O ----
    def din(name, shape, dt):
        return nc.dram_tensor(name, shape, dt, kind="ExternalInput")

    xarow = din("xArow", [NPAD, C], f16)
    xbrow = din("xBrow", [NPAD, C], f16)
    xad16 = din("xad16", [C, NTPAD], f16)
    xbd16 = din("xbd16", [C, NTPAD], f16)
    xsa = din("xsa", [NT, C], f32)
    xsb = din("xsb", [NT, C], f32)
    wka_d = din("wka", [C, C], f16)
    wkb_d = din("wkb", [C, C], f16)
    wv_d = [din(f"wv{r}", [C, C], f16) for r in range(3)]
    bv_d = [din(f"bv{r}", [1, C], f16) for r in range(3)]
    wqa = din("wqa", [C, 256], f16)
    bqa = din("bqa", [1, 256], f16)
    wqb = din("wqb", [C, 128], f16)
    bqb = din("bqb", [1, 128], f16)
    waa = din("waa", [C, C], f16)
    baa = din("baa", [1, C], f16)
    wab = din("wab", [C, C], f16)
    bab = din("bab", [1, C], f16)
    iota_d = din("iota", [P, NCH * 128], f16)
    blkd_d = din("blkd", [C, H], f16)
    ones1_d = din("ones1", [1, C], f16)
    ident_d = din("ident", [P, P], f16)
    idx_d = []
    for r in range(3):
        idx_d.append((
            din(f"idxk{r}", [P, meta["tot16"][r]], i16),
            din(f"idxq{r}", [P, meta["tot16"][r]], i16),
            din(f"drel{r}", [P, meta["tot128"][r]], f16),
            din(f"drln{r}", [P, meta["tot128"][r]], f32),
        ))
    outA = nc.dram_tensor("outA", [NT, C], f32, kind="ExternalOutput")
    outB = nc.dram_tensor("outB", [NT, C], f32, kind="ExternalOutput")

    Rt = meta["Rt"]
    visits = meta["visits"]
    betaA, betaB = meta["beta"]

    with tile.TileContext(nc) as tc:
        with tc.tile_pool(name="dram", bufs=1, space="DRAM") as dram:
            qra = dram.tile([NTPAD, 256], f16)
            qrb = dram.tile([NTPAD, 128], f16)

            with tc.tile_pool(name="const", bufs=1) as cp:
                iota_sb = cp.tile([P, NCH * 128], f16)
                nc.sync.dma_start(iota_sb[:], iota_d[:])
                blkd_sb = cp.tile([C, H], f16)
                nc.sync.dma_start(blkd_sb[:], blkd_d[:])
                ones1_sb = cp.tile([1, C], f16)
                nc.sync.dma_start(ones1_sb[:], ones1_d[:])
                ident_sb = cp.tile([P, P], f16)
                nc.sync.dma_start(ident_sb[:], ident_d[:])
                w_sb = {}
                src_map = {"wka": wka_d, "wkb": wkb_d,
                           "wv0": wv_d[0], "wv1": wv_d[1], "wv2": wv_d[2],
                           "bv0": bv_d[0], "bv1": bv_d[1], "bv2": bv_d[2],
                           "wqa": wqa, "bqa": bqa, "wqb": wqb, "bqb": bqb,
                           "waa": waa, "baa": baa, "wab": wab, "bab": bab}
                for nm, dt_, sh in [("wka", f16, [C, C]), ("wkb", f16, [C, C]),
                                    ("wv0", f16, [C, C]), ("wv1", f16, [C, C]),
                                    ("wv2", f16, [C, C]),
                                    ("bv0", f16, [1, C]), ("bv1", f16, [1, C]),
                                    ("bv2", f16, [1, C]),
                                    ("wqa", f16, [C, 256]), ("bqa", f16, [1, 256]),
                                    ("wqb", f16, [C, 128]), ("bqb", f16, [1, 128]),
                                    ("waa", f16, [C, C]), ("baa", f16, [1, C]),
                                    ("wab", f16, [C, C]), ("bab", f16, [1, C])]:
                    t = cp.tile(sh, dt_, tag=nm)
                    nc.sync.dma_start(t[:], src_map[nm][:])
                    w_sb[nm] = t

                import os as _osr
                _REP = int(_osr.environ.get("KERNEL_REPEAT", "1"))
                for _rep in range(_REP):
                    # ================= PHASE 1: build tables =================
                    with tc.tile_pool(name="prep", bufs=4) as pp, \
                         tc.tile_pool(name="prep_ps", bufs=3, space="PSUM") as pps:

                        def build_table(xdram, nrows, w, b, tbl, ncols):
                            GRP = 4  # node-tiles per DMA batch
                            ntiles = nrows // 128
                            base = 0
                            ii = 0
                            while base < ntiles:
                                grp = min(GRP, ntiles - base)
                                r0 = base * 128
                                xT = pp.tile([P, GRP, P], f16, tag="xT")
                                nc.sync.dma_start(
                                    xT[:, :grp, :], xdram[:, r0:r0 + grp * 128]
                                    .rearrange("c (t n) -> c t n", t=grp))
                                sb = pp.tile([P, GRP, ncols], f16, tag=f"sb{ncols}")
                                for t in range(grp):
                                    ps = pps.tile([P, 384], f32, tag="ps")
                                    nc.tensor.matmul(ps[:, :ncols], xT[:, t, :], w[:],
                                                     start=True, stop=False)
                                    nc.tensor.matmul(ps[:, :ncols], ones1_sb[:], b[:],
                                                     start=False, stop=True)
                                    if ii % 2 == 0:
                                        nc.scalar.copy(sb[:, t, :], ps[:, :ncols])
                                    else:
                                        nc.vector.tensor_copy(sb[:, t, :], ps[:, :ncols])
                                    ii += 1
                                nc.scalar.dma_start(
                                    tbl[r0:r0 + grp * 128, :]
                                    .rearrange("(t n) c -> n t c", n=128), sb[:, :grp, :])
                                base += grp

                        import os as _os1
                        if not _os1.environ.get("KERNEL_NOPREP"):
                            build_table(xad16, NTPAD, w_sb["wqa"], w_sb["bqa"], qra, 256)
                            build_table(xbd16, NTPAD, w_sb["wqb"], w_sb["bqb"], qrb, 128)

                    # ================= PHASE 2: streaming =================
                    with tc.tile_pool(name="agg", bufs=1) as apool:
                        agg = apool.tile([P, NSC * NCH, C], f32)

                        def out_stage(t):
                            import os as _os2
                            if _os2.environ.get("KERNEL_NOOUT"):
                                return
                            xs_d = xsa if t == 0 else xsb
                            out_d = outA if t == 0 else outB
                            wa = w_sb["waa" if t == 0 else "wab"]
                            bb = w_sb["baa" if t == 0 else "bab"]
                            bt = betaA if t == 0 else betaB
                            with tc.tile_pool(name="op", bufs=4) as op, \
                                 tc.tile_pool(name="ops", bufs=2, space="PSUM") as ops:
                                for slot in range(NSC * NCH):
                                    sc, ch = divmod(slot, NCH)
                                    rows = 98 if ch == 9 else 128
                                    base = sc * SCN + ch * 128
                                    g16 = op.tile([P, C], f16, tag="g16")
                                    nc.scalar.activation(g16[:], agg[:, slot, :], AF.Gelu)
                                    gt = ops.tile([P, C], f16, tag="gt")
                                    nc.tensor.transpose(gt[:], g16[:], ident_sb[:])
                                    gts = op.tile([P, C], f16, tag="gts")
                                    nc.vector.tensor_copy(gts[:], gt[:])
                                    o_ps = ops.tile([P, C], f32, tag="o")
                                    nc.tensor.matmul(o_ps[:], gts[:], wa[:], start=True, stop=False)
                                    nc.tensor.matmul(o_ps[:], ones1_sb[:], bb[:], start=False, stop=True)
                                    xs = op.tile([P, C], f32, tag="xs")
                                    nc.sync.dma_start(xs[:rows, :], xs_d[base:base + rows, :])
                                    ob = op.tile([P, C], f32, tag="ob")
                                    nc.scalar.activation(ob[:], o_ps[:], AF.Copy, scale=float(bt))
                                    res = op.tile([P, C], f32, tag="res")
                                    nc.vector.tensor_add(res[:rows, :], ob[:rows, :], xs[:rows, :])
                                    nc.sync.dma_start(out_d[base:base + rows, :], res[:rows, :])

                        with tc.tile_pool(name="gidx", bufs=1) as gi, \
                             tc.tile_pool(name="gp", bufs=2) as gp, \
                             tc.tile_pool(name="ep", bufs=4) as ep:
                            import os as _os
                            n_rel = int(_os.environ.get("KERNEL_NREL", "3"))
                            for r, (ekey, styp, dtyp) in enumerate(RELS[:n_rel]):
                                xrow = xarow if styp == 0 else xbrow
                                wk_sb = w_sb["wka" if styp == 0 else "wkb"]
                                wv_sb = w_sb[f"wv{r}"]
                                bv_sb = w_sb[f"bv{r}"]
                                if r == 0:
                                    qtab, qw, qoff = qrb, 128, 0
                                elif r == 1:
                                    qtab, qw, qoff = qra, 256, 0
                                else:
                                    qtab, qw, qoff = qra, 256, 128
                                qap = qtab[:, qoff:qoff + 128]

                                idxk_sb = gi.tile([P, meta["tot16"][r]], i16, tag="idxk")
                                nc.sync.dma_start(idxk_sb[:], idx_d[r][0][:])
                                idxq_sb = gi.tile([P, meta["tot16"][r]], i16, tag="idxq")
                                nc.sync.dma_start(idxq_sb[:], idx_d[r][1][:])
                                drel_sb = gi.tile([P, meta["tot128"][r]], f16, tag="drel")
                                nc.sync.dma_start(drel_sb[:], idx_d[r][2][:])

                                with tc.tile_pool(name=f"agps{r}", bufs=1, space="PSUM") as agps, \
                                     tc.tile_pool(name=f"kps{r}", bufs=1, space="PSUM") as kpool, \
                                     tc.tile_pool(name=f"vps{r}", bufs=2, space="PSUM") as vpool, \
                                     tc.tile_pool(name=f"lps{r}", bufs=1, space="PSUM") as lps:
                                    off16 = 0
                                    off128 = 0
                                    for sc in range(NSC):
                                        ag = agps.tile([P, 4, 512], f32, tag="aggps")
                                        for su in range(SUBT):
                                            R = Rt[r][sc][su]
                                            B = R // 128
                                            xap = xrow[su * SUBN:(su + 1) * SUBN, :]
                                            GC = 896  # per-gather idx cap (desc carveout is 1024)
                                            xT = gp.tile([P, 1, R], f16, tag="xT")
                                            for j0 in range(0, R, GC):
                                                n = min(GC, R - j0)
                                                nc.gpsimd.dma_gather(
                                                    xT[:, :, j0:j0 + n], xap,
                                                    idxk_sb[:, off16 + j0 // 16:off16 + (j0 + n) // 16],
                                                    n, n, 128, elem_step=128, transpose=True, queue_num=0)
                                            qT = gp.tile([P, 1, R], f16, tag="qT")
                                            for j0 in range(0, R, GC):
                                                n = min(GC, R - j0)
                                                nc.gpsimd.dma_gather(
                                                    qT[:, :, j0:j0 + n], qap,
                                                    idxq_sb[:, off16 + j0 // 16:off16 + (j0 + n) // 16],
                                                    n, n, 128, elem_step=qw, transpose=True, queue_num=1)
                                            # k^T = Wk^T @ x^T, staged through PSUM, copied to fp16
                                            kTs = gp.tile([P, R], f16, tag="kTs")
                                            for j0 in range(0, R, 512):
                                                n = min(512, R - j0)
                                                kps = kpool.tile([P, 512], f32, tag="kps")
                                                nc.tensor.matmul(kps[:, :n], wk_sb[:], xT[:, 0, j0:j0 + n],
                                                                 start=True, stop=True)
                                                nc.scalar.copy(kTs[:, j0:j0 + n], kps[:, :n])
                                            prod = gp.tile([P, R], f16, tag="prod")
                                            if not _os.environ.get("KERNEL_NOPROD"):
                                                nc.vector.tensor_mul(prod[:], kTs[:], qT[:, 0, :])
                                            prod_ap = prod[:]
                                            lpr = lps.tile([P, B, H], f32, tag="lp")
                                            for b in range(B):
                                                nc.tensor.matmul(lpr[:, b, :], prod_ap[:, b * 128:(b + 1) * 128],
                                                                 blkd_sb[:], start=(b == 0), stop=(b == B - 1))
                                            msg = gp.tile([P, B, 136], f16, tag="msg")
                                            nc.scalar.activation(msg[:, :, 128:136], lpr[:], AF.Exp)
                                            # vr = x @ (Wv blkM) + bv blkM, 4-chunk PSUM waves
                                            W = 4
                                            for w0 in range(0, B, W):
                                                wb = min(W, B - w0)
                                                vps = vpool.tile([P, W, 128], f32, tag="vps")
                                                for b in range(w0, w0 + wb):
                                                    nc.tensor.matmul(vps[:, b - w0, :],
                                                                     xT[:, 0, b * 128:(b + 1) * 128],
                                                                     wv_sb[:], start=True, stop=False)
                                                    nc.tensor.matmul(vps[:, b - w0, :],
                                                                     ones1_sb[:, 0:128], bv_sb[:],
                                                                     start=False, stop=True)
                                                if not _os.environ.get("KERNEL_NOMSGMUL"):
                                                    nc.vector.tensor_tensor(
                                                        out=msg[:, w0:w0 + wb, 0:128]
                                                        .rearrange("p b (h d) -> p b h d", d=DH),
                                                        in0=vps[:, :wb, :]
                                                        .rearrange("p b (h d) -> p b h d", d=DH),
                                                        in1=msg[:, w0:w0 + wb, 128:136]
                                                        .to_broadcast([P, wb, H, DH]),
                                                        op=ALU.mult)
                                                for b in range(w0, w0 + wb):
                                                    vlist = visits[r][sc][su][b]
                                                    if vlist and not _os.environ.get("KERNEL_NOONEHOT"):
                                                        c0 = min(ch for ch, _, _ in vlist)
                                                        c1 = max(ch for ch, _, _ in vlist)
                                                        ohw = gp.tile([P, NCH * 128], f16, tag="ohw")
                                                        nc.vector.tensor_tensor(
                                                            out=ohw[:, c0 * 128:(c1 + 1) * 128],
                                                            in0=iota_sb[:, c0 * 128:(c1 + 1) * 128],
                                                            in1=drel_sb[:, off128 + b:off128 + b + 1]
                                                            .to_broadcast([P, (c1 + 1 - c0) * 128]),
                                                            op=ALU.is_equal)
                                                    for (ch, ast, asp) in vlist:
                                                        oh_ap = (ohw[:, ch * 128:(ch + 1) * 128]
                                                                 if not _os.environ.get("KERNEL_NOONEHOT")
                                                                 else iota_sb[:, ch * 128:(ch + 1) * 128])
                                                        bk_, col = divmod(ch, 3)
                                                        nc.tensor.matmul(
                                                            ag[:, bk_, col * 136:col * 136 + 136],
                                                            oh_ap, msg[:, b, :], start=ast, stop=asp)
                                            off16 += R // 16
                                            off128 += B
                                        # epilogue for this superchunk
                                        for ch in range(NCH):
                                            bk_, col = divmod(ch, 3)
                                            a_ap = ag[:, bk_, col * 136:col * 136 + 128]
                                            s_ap = ag[:, bk_, col * 136 + 128:col * 136 + 136]
                                            rec = ep.tile([P, H], f32, tag="rec")
                                            nc.vector.tensor_scalar(rec[:], s_ap, 1e-16, None, op0=ALU.add)
                                            rec2 = ep.tile([P, H], f32, tag="rec2")
                                            nc.vector.reciprocal(rec2[:], rec[:])
                                            slot = sc * NCH + ch
                                            tgt = agg[:, slot, :].rearrange("p (h d) -> p h d", d=DH)
                                            src_v = a_ap.rearrange("p (h d) -> p h d", d=DH)
                                            if r == 2:
                                                tmp = ep.tile([P, C], f32, tag="tmp")
                                                nc.vector.tensor_tensor(
                                                    out=tmp[:].rearrange("p (h d) -> p h d", d=DH),
                                                    in0=src_v, in1=rec2[:].to_broadcast([P, H, DH]),
                                                    op=ALU.mult)
                                                nc.vector.tensor_add(agg[:, slot, :], agg[:, slot, :], tmp[:])
                                            else:
                                                nc.vector.tensor_tensor(
                                                    out=tgt, in0=src_v,
                                                    in1=rec2[:].to_broadcast([P, H, DH]),
                                                    op=ALU.mult)
                                if r == 0:
                                    out_stage(1)
                            out_stage(0)
    nc.compile()
    return nc


def _meta_key(meta):
    import json
    return json.dumps(meta, sort_keys=True)


def kernel(**inputs):
    meta, per_core = _host_prep(inputs)
    key = _meta_key(meta)
    if key not in _CACHE:
        _CACHE.clear()
        _CACHE[key] = _build_nc(meta)
    nc = _CACHE[key]

    from concourse.bass_utils import run_bass_kernel_spmd
    import os
    trace = bool(int(os.environ.get("KERNEL_TRACE", "0")))
    res = run_bass_kernel_spmd(nc, per_core, core_ids=list(range(M)), trace=trace)
    if trace:
        kernel.last_exec_time_ns = res.exec_time_ns
        kernel.last_trace = res.instructions_and_trace
    outs = res.results
    outA = np.concatenate([outs[m]["outA"] for m in range(M)], axis=0)
    outB = np.concatenate([outs[m]["outB"] for m in range(M)], axis=0)
    return np.stack([outA, outB]).astype(np.float32)



# revision 31
# speedup vs baseline: 1.3628x; 1.0656x over previous
"""Trainium2 Bass kernel for DragonHGT (heterogeneous graph transformer layer).

Strategy (8 NeuronCores, no collectives):
  - Shard edges by DESTINATION node range: core i owns dst nodes [i*12500, (i+1)*12500)
    of both node types. All segment ops (softmax denom, aggregation) become core-local.
  - Fold per-relation transforms into host-fused weights:
      qr = q @ a_rel^T * scale * p_rel   (folded into Wqr per relation, dst-side)
      vr = v @ m_rel                     (folded into Wvr per relation, src-side)
    so logits = <qr[dst], k[src]> per head and messages need no per-edge small matmuls.
  - Skip segment-max (logits are O(6) here; exp is safe in fp32/fp16 range) and
    normalize AFTER aggregation: agg = (sum_e e_e * vr_src) / (sum_e e_e).
  - Host bucket-sorts edges by (core, superchunk-of-1250-dst, src-subtable-of-25k, dst)
    so that src gathers use int16 dma_gather (fast SWDGE path) and the segment-sum
    is a one-hot matmul into PSUM per 128-node chunk.
  - Tables (k | vr...) are built on-device (replicated across cores) as fp16 HBM
    tables, then gathered per edge with dma_gather (k,qr transposed; vr plain).
"""

import math

import numpy as np

P = 128
NN = 100000          # nodes per type
C = 128
H = 8
DH = 16
M = 8                # cores
NT = NN // M         # 12500 dst rows per core
SCN = 1250           # dst nodes per superchunk
NSC = NT // SCN      # 10 superchunks per core
NCH = 10             # 128-node chunks per superchunk (9*128 + 98)
SUBT = 4             # src subtables
SUBN = NN // SUBT    # 25000
NPAD = 782 * 128     # 100096 (full tables padded)
NTPAD = 98 * 128     # 12544  (dst tables padded)
SCALE = 1.0 / math.sqrt(DH)

# relations: (edge_key, src_type, dst_type)
RELS = [("eAB", 0, 1), ("eBA", 1, 0), ("eAA", 0, 0)]

_CACHE = {}


def _sigmoid(x):
    return 1.0 / (1.0 + np.exp(-x))


def _blockdiag(mats):
    """mats: [H, DH, DH] -> [C, C] block diagonal."""
    out = np.zeros((C, C), np.float32)
    for h in range(H):
        out[h * DH:(h + 1) * DH, h * DH:(h + 1) * DH] = mats[h]
    return out


def _wrap16(arr_i16):
    """[R] int16 -> [128, R//16] wrapped (idx j at [j%16, j//16]) replicated to 128 partitions."""
    R = arr_i16.shape[0]
    w = arr_i16.reshape(R // 16, 16).T  # [16, R/16]
    return np.tile(w, (8, 1))


def _wrap128(arr):
    """[R] -> [128, R//128] (edge j at [j%128, j//128])."""
    R = arr.shape[0]
    return np.ascontiguousarray(arr.reshape(R // 128, 128).T)


def _host_prep(inputs):
    """Returns (meta, per_core_inputs). meta is SPMD-identical; arrays differ per core."""
    xA = np.asarray(inputs["xA"], np.float32)
    xB = np.asarray(inputs["xB"], np.float32)
    Wk = np.asarray(inputs["Wk"], np.float32)
    bk = np.asarray(inputs["bk"], np.float32)
    Wq = np.asarray(inputs["Wq"], np.float32)
    bq = np.asarray(inputs["bq"], np.float32)
    Wv = np.asarray(inputs["Wv"], np.float32)
    bv = np.asarray(inputs["bv"], np.float32)
    Wa = np.asarray(inputs["Wa"], np.float32)
    ba = np.asarray(inputs["ba"], np.float32)
    skip = np.asarray(inputs["skip"], np.float32)
    a_rel = np.asarray(inputs["a_rel"], np.float32)
    m_rel = np.asarray(inputs["m_rel"], np.float32)
    p_rel = np.asarray(inputs["p_rel"], np.float32)

    beta = _sigmoid(skip)  # [2]

    # ---- fused weights ----
    # On-the-fly derivation: per edge gather x[src] once; k = x @ Wk (bk dropped
    # exactly: a per-(dst,head) logit shift cancels in the softmax ratio),
    # vr = x @ (Wv @ blkM[r]) + bv @ blkM[r] (bias added via ones-outer matmul).
    blkM = [_blockdiag(m_rel[r]) for r in range(3)]
    wka, wkb = Wk[0], Wk[1]
    wvr = [Wv[(0, 1, 0)[r]] @ blkM[r] for r in range(3)]
    bvr = [bv[(0, 1, 0)[r]] @ blkM[r] for r in range(3)]
    # qr weights: qr_r = q_t(r) @ blkdiag(a_rel[r].T) * scale * p_rel[r,h]
    blkQ = []
    for r in range(3):
        mats = [a_rel[r, h].T * (SCALE * p_rel[r, h]) for h in range(H)]
        blkQ.append(_blockdiag(np.stack(mats)))
    # dst types: rel0 -> B, rel1 -> A, rel2 -> A
    wqb = Wq[1] @ blkQ[0]
    bqb = bq[1] @ blkQ[0]
    wqa = np.concatenate([Wq[0] @ blkQ[1], Wq[0] @ blkQ[2]], axis=1)  # [128,256]
    bqa = np.concatenate([bq[0] @ blkQ[1], bq[0] @ blkQ[2]])

    # ---- consts ----
    iota = np.tile(np.arange(SCN + 30, dtype=np.float32)[None, :NCH * 128], (P, 1)).astype(np.float16)
    blkd = np.zeros((C, H), np.float16)
    for h in range(H):
        blkd[h * DH:(h + 1) * DH, h] = 1.0
    ones1 = np.ones((1, C), np.float16)
    ident = np.eye(P, dtype=np.float16)

    # ---- per-type padded fp16 x ----
    def pad_rows(a, n):
        out = np.zeros((n, a.shape[1]), a.dtype)
        out[: a.shape[0]] = a
        return out

    xArow = pad_rows(xA.astype(np.float16), NPAD)  # [NPAD, C] row-major gather table
    xBrow = pad_rows(xB.astype(np.float16), NPAD)

    # ---- edge prep ----
    meta = {"Rt": [], "visits": [], "tot16": [], "tot128": []}
    per_core = [dict() for _ in range(M)]
    rng_extra = 0
    for ri, (ekey, styp, dtyp) in enumerate(RELS):
        e = np.asarray(inputs[ekey])
        src = e[0].astype(np.int64)
        dst = e[1].astype(np.int64)
        core = dst // NT
        scid = (dst % NT) // SCN
        sub = src // SUBN
        key = (core * NSC + scid) * SUBT + sub
        order = np.lexsort((dst, key))
        src_s = src[order]
        dst_s = dst[order]
        key_s = key[order]
        counts = np.bincount(key_s, minlength=M * NSC * SUBT).reshape(M, NSC, SUBT)
        Rt = np.maximum(128, ((counts.max(axis=0) + 127) // 128) * 128)  # [NSC, SUBT]
        starts = np.zeros(M * NSC * SUBT + 1, np.int64)
        np.cumsum(counts.reshape(-1), out=starts[1:])

        # per-core arrays + per-batch chunk spans
        tot16 = int(Rt.sum() // 16)
        tot128 = int(Rt.sum() // 128)
        spans = {}  # (sc, sub, b) -> [cmin, cmax] union over cores
        for m in range(M):
            ik = np.zeros(int(Rt.sum()), np.int16)
            iq = np.zeros(int(Rt.sum()), np.int16)
            dr = np.full(int(Rt.sum()), -1.0, np.float16)
            off = 0
            for sc in range(NSC):
                for su in range(SUBT):
                    R = int(Rt[sc, su])
                    k = (m * NSC + sc) * SUBT + su
                    lo, hi = int(starts[k]), int(starts[k + 1])
                    n = hi - lo
                    ik[off:off + n] = (src_s[lo:hi] - su * SUBN).astype(np.int16)
                    iq[off:off + n] = (dst_s[lo:hi] - m * NT).astype(np.int16)
                    dl = (dst_s[lo:hi] - m * NT - sc * SCN).astype(np.int32)
                    dr[off:off + n] = dl.astype(np.float16)
                    for b in range(R // 128):
                        if b * 128 >= n:
                            break
                        c0 = int(dl[b * 128]) // 128
                        c1 = int(dl[min(b * 128 + 127, n - 1)]) // 128
                        kk = (sc, su, b)
                        if kk in spans:
                            spans[kk][0] = min(spans[kk][0], c0)
                            spans[kk][1] = max(spans[kk][1], c1)
                        else:
                            spans[kk] = [c0, c1]
                    off += R
            per_core[m][f"idxk{ri}"] = _wrap16(ik)
            per_core[m][f"idxq{ri}"] = _wrap16(iq)
            per_core[m][f"drel{ri}"] = _wrap128(dr)
            per_core[m][f"drln{ri}"] = _wrap128((-dr).astype(np.float32))

        # build visit lists with per-BANK psum group start/stop flags (PSUM zero
        # regions are 2KB = one bank; only one accumulation group per bank, and
        # start zeroes the whole bank). One fused [msg|e8] matmul per visit:
        # chunk c -> bank c//3, offset (c%3)*136 (128 msg cols + 8 exp-sum cols).
        visits = []  # [sc][sub][b] -> list of (chunk, start, stop)
        for sc in range(NSC):
            order_v = []  # (sub, b, chunk) program order
            for su in range(SUBT):
                for b in range(int(Rt[sc, su]) // 128):
                    sp = spans.get((sc, su, b))
                    if sp is None:
                        continue
                    for c in range(sp[0], sp[1] + 1):
                        order_v.append((su, b, c))
            seen_banks = set(c // 3 for _, _, c in order_v)
            last_su = SUBT - 1
            last_b = int(Rt[sc, last_su]) // 128 - 1
            for bk in range(4):
                if bk not in seen_banks:
                    order_v.append((last_su, last_b, bk * 3))
            first = {}
            last = {}
            for j, (_, _, c) in enumerate(order_v):
                bk = c // 3
                if bk not in first:
                    first[bk] = j
                last[bk] = j
            vl = [[[] for _ in range(int(Rt[sc, su]) // 128)] for su in range(SUBT)]
            for j, (su, b, c) in enumerate(order_v):
                bk = c // 3
                vl[su][b].append((c, j == first[bk], j == last[bk]))
            visits.append(vl)

        meta["Rt"].append([[int(x) for x in row] for row in Rt])
        meta["visits"].append(visits)
        meta["tot16"].append(tot16)
        meta["tot128"].append(tot128)
        rng_extra += int(Rt.sum())

    meta["beta"] = [float(beta[0]), float(beta[1])]
    meta["zbv"] = [bool(np.all(bvr[r] == 0)) for r in range(3)]
    meta["zba"] = [bool(np.all(ba[t] == 0)) for t in (0, 1)]
    meta["zbqa"] = bool(np.all(bqa == 0))
    meta["zbqb"] = bool(np.all(bqb == 0))

    # ---- shared (replicated) inputs ----
    shared = {
        "xArow": xArow, "xBrow": xBrow,
        "wka": wka.astype(np.float16), "wkb": wkb.astype(np.float16),
        "wv0": wvr[0].astype(np.float16), "wv1": wvr[1].astype(np.float16),
        "wv2": wvr[2].astype(np.float16),
        "bv0": bvr[0].astype(np.float16)[None, :], "bv1": bvr[1].astype(np.float16)[None, :],
        "bv2": bvr[2].astype(np.float16)[None, :],
        "wqa": wqa.astype(np.float16), "bqa": bqa.astype(np.float16)[None, :],
        "wqb": wqb.astype(np.float16), "bqb": bqb.astype(np.float16)[None, :],
        "waa": Wa[0].astype(np.float16), "baa": ba[0].astype(np.float16)[None, :],
        "wab": Wa[1].astype(np.float16), "bab": ba[1].astype(np.float16)[None, :],
        "iota": iota, "blkd": blkd, "ones1": ones1, "ident": ident,
    }
    for m in range(M):
        r0, r1 = m * NT, (m + 1) * NT
        per_core[m]["xad16"] = np.ascontiguousarray(pad_rows(xA[r0:r1].astype(np.float16), NTPAD).T)
        per_core[m]["xbd16"] = np.ascontiguousarray(pad_rows(xB[r0:r1].astype(np.float16), NTPAD).T)
        per_core[m]["xsa"] = np.ascontiguousarray((1.0 - beta[0]) * xA[r0:r1])
        per_core[m]["xsb"] = np.ascontiguousarray((1.0 - beta[1]) * xB[r0:r1])
        per_core[m].update(shared)
    return meta, per_core


def _build_nc(meta):
    import concourse.bacc as bacc
    import concourse.mybir as mybir
    import concourse.tile as tile

    f16 = mybir.dt.float16
    f32 = mybir.dt.float32
    i16 = mybir.dt.int16
    AF = mybir.ActivationFunctionType
    ALU = mybir.AluOpType

    nc = bacc.Bacc("TRN2", target_bir_lowering=False, debug=False, num_swdge_queues=4)

    # ---- I/O ----
    def din(name, shape, dt):
        return nc.dram_tensor(name, shape, dt, kind="ExternalInput")

    xarow = din("xArow", [NPAD, C], f16)
    xbrow = din("xBrow", [NPAD, C], f16)
    xad16 = din("xad16", [C, NTPAD], f16)
    xbd16 = din("xbd16", [C, NTPAD], f16)
    xsa = din("xsa", [NT, C], f32)
    xsb = din("xsb", [NT, C], f32)
    wka_d = din("wka", [C, C], f16)
    wkb_d = din("wkb", [C, C], f16)
    wv_d = [din(f"wv{r}", [C, C], f16) for r in range(3)]
    bv_d = [din(f"bv{r}", [1, C], f16) for r in range(3)]
    wqa = din("wqa", [C, 256], f16)
    bqa = din("bqa", [1, 256], f16)
    wqb = din("wqb", [C, 128], f16)
    bqb = din("bqb", [1, 128], f16)
    waa = din("waa", [C, C], f16)
    baa = din("baa", [1, C], f16)
    wab = din("wab", [C, C], f16)
    bab = din("bab", [1, C], f16)
    iota_d = din("iota", [P, NCH * 128], f16)
    blkd_d = din("blkd", [C, H], f16)
    ones1_d = din("ones1", [1, C], f16)
    ident_d = din("ident", [P, P], f16)
    idx_d = []
    for r in range(3):
        idx_d.append((
            din(f"idxk{r}", [P, meta["tot16"][r]], i16),
            din(f"idxq{r}", [P, meta["tot16"][r]], i16),
            din(f"drel{r}", [P, meta["tot128"][r]], f16),
            din(f"drln{r}", [P, meta["tot128"][r]], f32),
        ))
    outA = nc.dram_tensor("outA", [NT, C], f32, kind="ExternalOutput")
    outB = nc.dram_tensor("outB", [NT, C], f32, kind="ExternalOutput")

    Rt = meta["Rt"]
    visits = meta["visits"]
    betaA, betaB = meta["beta"]

    with tile.TileContext(nc) as tc:
        with tc.tile_pool(name="dram", bufs=1, space="DRAM") as dram:
            qra = dram.tile([NTPAD, 256], f16)
            qrb = dram.tile([NTPAD, 128], f16)

            with tc.tile_pool(name="const", bufs=1) as cp:
                iota_sb = cp.tile([P, NCH * 128], f16)
                nc.sync.dma_start(iota_sb[:], iota_d[:])
                blkd_sb = cp.tile([C, H], f16)
                nc.sync.dma_start(blkd_sb[:], blkd_d[:])
                ones1_sb = cp.tile([1, C], f16)
                nc.sync.dma_start(ones1_sb[:], ones1_d[:])
                ident_sb = cp.tile([P, P], f16)
                nc.sync.dma_start(ident_sb[:], ident_d[:])
                w_sb = {}
                src_map = {"wka": wka_d, "wkb": wkb_d,
                           "wv0": wv_d[0], "wv1": wv_d[1], "wv2": wv_d[2],
                           "bv0": bv_d[0], "bv1": bv_d[1], "bv2": bv_d[2],
                           "wqa": wqa, "bqa": bqa, "wqb": wqb, "bqb": bqb,
                           "waa": waa, "baa": baa, "wab": wab, "bab": bab}
                for nm, dt_, sh in [("wka", f16, [C, C]), ("wkb", f16, [C, C]),
                                    ("wv0", f16, [C, C]), ("wv1", f16, [C, C]),
                                    ("wv2", f16, [C, C]),
                                    ("bv0", f16, [1, C]), ("bv1", f16, [1, C]),
                                    ("bv2", f16, [1, C]),
                                    ("wqa", f16, [C, 256]), ("bqa", f16, [1, 256]),
                                    ("wqb", f16, [C, 128]), ("bqb", f16, [1, 128]),
                                    ("waa", f16, [C, C]), ("baa", f16, [1, C]),
                                    ("wab", f16, [C, C]), ("bab", f16, [1, C])]:
                    t = cp.tile(sh, dt_, tag=nm)
                    nc.sync.dma_start(t[:], src_map[nm][:])
                    w_sb[nm] = t

                import os as _osr
                _REP = int(_osr.environ.get("KERNEL_REPEAT", "1"))
                for _rep in range(_REP):
                    # ================= PHASE 1: build tables =================
                    with tc.tile_pool(name="prep", bufs=4) as pp, \
                         tc.tile_pool(name="prep_ps", bufs=3, space="PSUM") as pps:

                        def build_table(xdram, nrows, w, b, tbl, ncols, skip_bias):
                            GRP = 4  # node-tiles per DMA batch
                            ntiles = nrows // 128
                            base = 0
                            ii = 0
                            while base < ntiles:
                                grp = min(GRP, ntiles - base)
                                r0 = base * 128
                                xT = pp.tile([P, GRP, P], f16, tag="xT")
                                nc.sync.dma_start(
                                    xT[:, :grp, :], xdram[:, r0:r0 + grp * 128]
                                    .rearrange("c (t n) -> c t n", t=grp))
                                sb = pp.tile([P, GRP, ncols], f16, tag=f"sb{ncols}")
                                for t in range(grp):
                                    ps = pps.tile([P, 384], f32, tag="ps")
                                    nc.tensor.matmul(ps[:, :ncols], xT[:, t, :], w[:],
                                                     start=True, stop=skip_bias)
                                    if not skip_bias:
                                        nc.tensor.matmul(ps[:, :ncols], ones1_sb[:], b[:],
                                                         start=False, stop=True)
                                    if ii % 2 == 0:
                                        nc.scalar.copy(sb[:, t, :], ps[:, :ncols])
                                    else:
                                        nc.vector.tensor_copy(sb[:, t, :], ps[:, :ncols])
                                    ii += 1
                                nc.scalar.dma_start(
                                    tbl[r0:r0 + grp * 128, :]
                                    .rearrange("(t n) c -> n t c", n=128), sb[:, :grp, :])
                                base += grp

                        import os as _os1
                        if not _os1.environ.get("KERNEL_NOPREP"):
                            build_table(xad16, NTPAD, w_sb["wqa"], w_sb["bqa"], qra, 256,
                                        meta["zbqa"])
                            build_table(xbd16, NTPAD, w_sb["wqb"], w_sb["bqb"], qrb, 128,
                                        meta["zbqb"])

                    # ================= PHASE 2: streaming =================
                    with tc.tile_pool(name="agg", bufs=1) as apool:
                        agg = apool.tile([P, NSC * NCH, C], f16)

                        def out_stage(t):
                            import os as _os2
                            if _os2.environ.get("KERNEL_NOOUT"):
                                return
                            xs_d = xsa if t == 0 else xsb
                            out_d = outA if t == 0 else outB
                            wa = w_sb["waa" if t == 0 else "wab"]
                            bb = w_sb["baa" if t == 0 else "bab"]
                            bt = betaA if t == 0 else betaB
                            with tc.tile_pool(name="op", bufs=4) as op, \
                                 tc.tile_pool(name="ops", bufs=2, space="PSUM") as ops:
                                for slot in range(NSC * NCH):
                                    sc, ch = divmod(slot, NCH)
                                    rows = 98 if ch == 9 else 128
                                    base = sc * SCN + ch * 128
                                    g16 = op.tile([P, C], f16, tag="g16")
                                    nc.scalar.activation(g16[:], agg[:, slot, :], AF.Gelu)
                                    gt = ops.tile([P, C], f16, tag="gt")
                                    nc.tensor.transpose(gt[:], g16[:], ident_sb[:])
                                    gts = op.tile([P, C], f16, tag="gts")
                                    nc.vector.tensor_copy(gts[:], gt[:])
                                    o_ps = ops.tile([P, C], f32, tag="o")
                                    zb = meta["zba"][t]
                                    nc.tensor.matmul(o_ps[:], gts[:], wa[:], start=True, stop=zb)
                                    if not zb:
                                        nc.tensor.matmul(o_ps[:], ones1_sb[:], bb[:],
                                                         start=False, stop=True)
                                    xs = op.tile([P, C], f32, tag="xs")
                                    nc.sync.dma_start(xs[:rows, :], xs_d[base:base + rows, :])
                                    ob = op.tile([P, C], f32, tag="ob")
                                    nc.scalar.activation(ob[:], o_ps[:], AF.Copy, scale=float(bt))
                                    res = op.tile([P, C], f32, tag="res")
                                    nc.vector.tensor_add(res[:rows, :], ob[:rows, :], xs[:rows, :])
                                    nc.sync.dma_start(out_d[base:base + rows, :], res[:rows, :])

                        with tc.tile_pool(name="gidx", bufs=1) as gi, \
                             tc.tile_pool(name="gp", bufs=2) as gp, \
                             tc.tile_pool(name="ep", bufs=4) as ep:
                            import os as _os
                            n_rel = int(_os.environ.get("KERNEL_NREL", "3"))
                            for r, (ekey, styp, dtyp) in enumerate(RELS[:n_rel]):
                                xrow = xarow if styp == 0 else xbrow
                                wk_sb = w_sb["wka" if styp == 0 else "wkb"]
                                wv_sb = w_sb[f"wv{r}"]
                                bv_sb = w_sb[f"bv{r}"]
                                if r == 0:
                                    qtab, qw, qoff = qrb, 128, 0
                                elif r == 1:
                                    qtab, qw, qoff = qra, 256, 0
                                else:
                                    qtab, qw, qoff = qra, 256, 128
                                qap = qtab[:, qoff:qoff + 128]

                                idxk_sb = gi.tile([P, meta["tot16"][r]], i16, tag="idxk")
                                nc.sync.dma_start(idxk_sb[:], idx_d[r][0][:])
                                idxq_sb = gi.tile([P, meta["tot16"][r]], i16, tag="idxq")
                                nc.sync.dma_start(idxq_sb[:], idx_d[r][1][:])
                                drel_sb = gi.tile([P, meta["tot128"][r]], f16, tag="drel")
                                nc.sync.dma_start(drel_sb[:], idx_d[r][2][:])

                                with tc.tile_pool(name=f"agps{r}", bufs=1, space="PSUM") as agps, \
                                     tc.tile_pool(name=f"kps{r}", bufs=1, space="PSUM") as kpool, \
                                     tc.tile_pool(name=f"vps{r}", bufs=2, space="PSUM") as vpool, \
                                     tc.tile_pool(name=f"lps{r}", bufs=1, space="PSUM") as lps:
                                    off16 = 0
                                    off128 = 0
                                    for sc in range(NSC):
                                        ag = agps.tile([P, 4, 512], f32, tag="aggps")
                                        for su in range(SUBT):
                                            R = Rt[r][sc][su]
                                            B = R // 128
                                            xap = xrow[su * SUBN:(su + 1) * SUBN, :]
                                            GC = 896  # per-gather idx cap (desc carveout is 1024)
                                            qpar = 2 * ((sc * SUBT + su) % 2)
                                            xT = gp.tile([P, 1, R], f16, tag="xT")
                                            for j0 in range(0, R, GC):
                                                n = min(GC, R - j0)
                                                nc.gpsimd.dma_gather(
                                                    xT[:, :, j0:j0 + n], xap,
                                                    idxk_sb[:, off16 + j0 // 16:off16 + (j0 + n) // 16],
                                                    n, n, 128, elem_step=128, transpose=True,
                                                    queue_num=qpar)
                                            qT = gp.tile([P, 1, R], f16, tag="qT")
                                            for j0 in range(0, R, GC):
                                                n = min(GC, R - j0)
                                                nc.gpsimd.dma_gather(
                                                    qT[:, :, j0:j0 + n], qap,
                                                    idxq_sb[:, off16 + j0 // 16:off16 + (j0 + n) // 16],
                                                    n, n, 128, elem_step=qw, transpose=True,
                                                    queue_num=qpar + 1)
                                            # k^T = Wk^T @ x^T, staged through PSUM, copied to fp16
                                            kTs = gp.tile([P, R], f16, tag="kTs")
                                            for j0 in range(0, R, 512):
                                                n = min(512, R - j0)
                                                kps = kpool.tile([P, 512], f32, tag="kps")
                                                nc.tensor.matmul(kps[:, :n], wk_sb[:], xT[:, 0, j0:j0 + n],
                                                                 start=True, stop=True)
                                                nc.scalar.copy(kTs[:, j0:j0 + n], kps[:, :n])
                                            prod = gp.tile([P, R], f16, tag="prod")
                                            if not _os.environ.get("KERNEL_NOPROD"):
                                                nc.vector.tensor_mul(prod[:], kTs[:], qT[:, 0, :])
                                            prod_ap = prod[:]
                                            lpr = lps.tile([P, B, H], f32, tag="lp")
                                            for b in range(B):
                                                nc.tensor.matmul(lpr[:, b, :], prod_ap[:, b * 128:(b + 1) * 128],
                                                                 blkd_sb[:], start=(b == 0), stop=(b == B - 1))
                                            msg = gp.tile([P, B, 136], f16, tag="msg")
                                            nc.scalar.activation(msg[:, :, 128:136], lpr[:], AF.Exp)
                                            # vr = x @ (Wv blkM) + bv blkM, 4-chunk PSUM waves
                                            W = 4
                                            for w0 in range(0, B, W):
                                                wb = min(W, B - w0)
                                                vps = vpool.tile([P, W, 128], f32, tag="vps")
                                                zb = meta["zbv"][r]
                                                for b in range(w0, w0 + wb):
                                                    nc.tensor.matmul(vps[:, b - w0, :],
                                                                     xT[:, 0, b * 128:(b + 1) * 128],
                                                                     wv_sb[:], start=True, stop=zb)
                                                    if not zb:
                                                        nc.tensor.matmul(vps[:, b - w0, :],
                                                                         ones1_sb[:, 0:128], bv_sb[:],
                                                                         start=False, stop=True)
                                                if not _os.environ.get("KERNEL_NOMSGMUL"):
                                                    nc.vector.tensor_tensor(
                                                        out=msg[:, w0:w0 + wb, 0:128]
                                                        .rearrange("p b (h d) -> p b h d", d=DH),
                                                        in0=vps[:, :wb, :]
                                                        .rearrange("p b (h d) -> p b h d", d=DH),
                                                        in1=msg[:, w0:w0 + wb, 128:136]
                                                        .to_broadcast([P, wb, H, DH]),
                                                        op=ALU.mult)
                                                for b in range(w0, w0 + wb):
                                                    vlist = visits[r][sc][su][b]
                                                    if vlist and not _os.environ.get("KERNEL_NOONEHOT"):
                                                        c0 = min(ch for ch, _, _ in vlist)
                                                        c1 = max(ch for ch, _, _ in vlist)
                                                        ohw = gp.tile([P, NCH * 128], f16, tag="ohw")
                                                        nc.vector.tensor_tensor(
                                                            out=ohw[:, c0 * 128:(c1 + 1) * 128],
                                                            in0=iota_sb[:, c0 * 128:(c1 + 1) * 128],
                                                            in1=drel_sb[:, off128 + b:off128 + b + 1]
                                                            .to_broadcast([P, (c1 + 1 - c0) * 128]),
                                                            op=ALU.is_equal)
                                                    for (ch, ast, asp) in vlist:
                                                        oh_ap = (ohw[:, ch * 128:(ch + 1) * 128]
                                                                 if not _os.environ.get("KERNEL_NOONEHOT")
                                                                 else iota_sb[:, ch * 128:(ch + 1) * 128])
                                                        bk_, col = divmod(ch, 3)
                                                        nc.tensor.matmul(
                                                            ag[:, bk_, col * 136:col * 136 + 136],
                                                            oh_ap, msg[:, b, :], start=ast, stop=asp)
                                            off16 += R // 16
                                            off128 += B
                                        # epilogue for this superchunk
                                        for ch in range(NCH):
                                            bk_, col = divmod(ch, 3)
                                            a_ap = ag[:, bk_, col * 136:col * 136 + 128]
                                            s_ap = ag[:, bk_, col * 136 + 128:col * 136 + 136]
                                            rec = ep.tile([P, H], f32, tag="rec")
                                            nc.vector.tensor_scalar(rec[:], s_ap, 1e-16, None, op0=ALU.add)
                                            rec2 = ep.tile([P, H], f32, tag="rec2")
                                            nc.vector.reciprocal(rec2[:], rec[:])
                                            slot = sc * NCH + ch
                                            tgt = agg[:, slot, :].rearrange("p (h d) -> p h d", d=DH)
                                            src_v = a_ap.rearrange("p (h d) -> p h d", d=DH)
                                            if r == 2:
                                                tmp = ep.tile([P, C], f16, tag="tmp")
                                                nc.vector.tensor_tensor(
                                                    out=tmp[:].rearrange("p (h d) -> p h d", d=DH),
                                                    in0=src_v, in1=rec2[:].to_broadcast([P, H, DH]),
                                                    op=ALU.mult)
                                                nc.vector.tensor_add(agg[:, slot, :], agg[:, slot, :], tmp[:])
                                            else:
                                                nc.vector.tensor_tensor(
                                                    out=tgt, in0=src_v,
                                                    in1=rec2[:].to_broadcast([P, H, DH]),
                                                    op=ALU.mult)
                                if r == 0:
                                    out_stage(1)
                            out_stage(0)
    nc.compile()
    return nc


def _meta_key(meta):
    import json
    return json.dumps(meta, sort_keys=True)


def kernel(**inputs):
    meta, per_core = _host_prep(inputs)
    key = _meta_key(meta)
    if key not in _CACHE:
        _CACHE.clear()
        _CACHE[key] = _build_nc(meta)
    nc = _CACHE[key]

    from concourse.bass_utils import run_bass_kernel_spmd
    import os
    trace = bool(int(os.environ.get("KERNEL_TRACE", "0")))
    res = run_bass_kernel_spmd(nc, per_core, core_ids=list(range(M)), trace=trace)
    if trace:
        kernel.last_exec_time_ns = res.exec_time_ns
        kernel.last_trace = res.instructions_and_trace
    outs = res.results
    outA = np.concatenate([outs[m]["outA"] for m in range(M)], axis=0)
    outB = np.concatenate([outs[m]["outB"] for m in range(M)], axis=0)
    return np.stack([outA, outB]).astype(np.float32)



# revision 44
# speedup vs baseline: 1.7470x; 1.2819x over previous
"""Trainium2 Bass kernel for DragonHGT (heterogeneous graph transformer layer).

Strategy (8 NeuronCores, no collectives):
  - Shard edges by DESTINATION node range: core i owns dst nodes [i*12500, (i+1)*12500)
    of both node types. All segment ops (softmax denom, aggregation) become core-local.
  - Fold per-relation transforms into host-fused weights:
      qr = q @ a_rel^T * scale * p_rel   (folded into Wqr per relation, dst-side)
      vr = v @ m_rel                     (folded into Wvr per relation, src-side)
    so logits = <qr[dst], k[src]> per head and messages need no per-edge small matmuls.
  - Skip segment-max (logits are O(6) here; exp is safe in fp32/fp16 range) and
    normalize AFTER aggregation: agg = (sum_e e_e * vr_src) / (sum_e e_e).
  - Host bucket-sorts edges by (core, superchunk-of-1250-dst, src-subtable-of-25k, dst)
    so that src gathers use int16 dma_gather (fast SWDGE path) and the segment-sum
    is a one-hot matmul into PSUM per 128-node chunk.
  - Tables (k | vr...) are built on-device (replicated across cores) as fp16 HBM
    tables, then gathered per edge with dma_gather (k,qr transposed; vr plain).
"""

import math

import numpy as np

P = 128
NN = 100000          # nodes per type
C = 128
H = 8
DH = 16
M = 8                # cores
NT = NN // M         # 12500 dst rows per core
SCN = 1250           # dst nodes per superchunk
NSC = NT // SCN      # 10 superchunks per core
NCH = 10             # 128-node chunks per superchunk (9*128 + 98)
SUBT = 4             # src subtables
SUBN = NN // SUBT    # 25000
NPAD = 782 * 128     # 100096 (full tables padded)
NTPAD = 98 * 128     # 12544  (dst tables padded)
SCALE = 1.0 / math.sqrt(DH)

# relations: (edge_key, src_type, dst_type)
RELS = [("eAB", 0, 1), ("eBA", 1, 0), ("eAA", 0, 0)]

_CACHE = {}


def _sigmoid(x):
    return 1.0 / (1.0 + np.exp(-x))


def _blockdiag(mats):
    """mats: [H, DH, DH] -> [C, C] block diagonal."""
    out = np.zeros((C, C), np.float32)
    for h in range(H):
        out[h * DH:(h + 1) * DH, h * DH:(h + 1) * DH] = mats[h]
    return out


def _wrap16(arr_i16):
    """[R] int16 -> [128, R//16] wrapped (idx j at [j%16, j//16]) replicated to 128 partitions."""
    R = arr_i16.shape[0]
    w = arr_i16.reshape(R // 16, 16).T  # [16, R/16]
    return np.tile(w, (8, 1))


def _wrap128(arr):
    """[R] -> [128, R//128] (edge j at [j%128, j//128])."""
    R = arr.shape[0]
    return np.ascontiguousarray(arr.reshape(R // 128, 128).T)


def _host_prep(inputs):
    """Returns (meta, per_core_inputs). meta is SPMD-identical; arrays differ per core."""
    xA = np.asarray(inputs["xA"], np.float32)
    xB = np.asarray(inputs["xB"], np.float32)
    Wk = np.asarray(inputs["Wk"], np.float32)
    bk = np.asarray(inputs["bk"], np.float32)
    Wq = np.asarray(inputs["Wq"], np.float32)
    bq = np.asarray(inputs["bq"], np.float32)
    Wv = np.asarray(inputs["Wv"], np.float32)
    bv = np.asarray(inputs["bv"], np.float32)
    Wa = np.asarray(inputs["Wa"], np.float32)
    ba = np.asarray(inputs["ba"], np.float32)
    skip = np.asarray(inputs["skip"], np.float32)
    a_rel = np.asarray(inputs["a_rel"], np.float32)
    m_rel = np.asarray(inputs["m_rel"], np.float32)
    p_rel = np.asarray(inputs["p_rel"], np.float32)

    beta = _sigmoid(skip)  # [2]

    # ---- fused weights ----
    # A-type src table: [ kA | vr(rel0) | vr(rel2) ]  (rel0: A->B, rel2: A->A)
    blkM = [_blockdiag(m_rel[r]) for r in range(3)]
    wfa = np.concatenate([Wk[0], Wv[0] @ blkM[0], Wv[0] @ blkM[2]], axis=1)  # [128,384]
    bfa = np.concatenate([bk[0], bv[0] @ blkM[0], bv[0] @ blkM[2]])          # [384]
    wfb = np.concatenate([Wk[1], Wv[1] @ blkM[1]], axis=1)                   # [128,256]
    bfb = np.concatenate([bk[1], bv[1] @ blkM[1]])
    # qr weights: qr_r = q_t(r) @ blkdiag(a_rel[r].T) * scale * p_rel[r,h]
    blkQ = []
    for r in range(3):
        mats = [a_rel[r, h].T * (SCALE * p_rel[r, h]) for h in range(H)]
        blkQ.append(_blockdiag(np.stack(mats)))
    # dst types: rel0 -> B, rel1 -> A, rel2 -> A
    wqb = Wq[1] @ blkQ[0]
    bqb = bq[1] @ blkQ[0]
    wqa = np.concatenate([Wq[0] @ blkQ[1], Wq[0] @ blkQ[2]], axis=1)  # [128,256]
    bqa = np.concatenate([bq[0] @ blkQ[1], bq[0] @ blkQ[2]])

    # ---- consts ----
    iota = np.tile(np.arange(SCN + 30, dtype=np.float32)[None, :NCH * 128], (P, 1)).astype(np.float16)
    blkd = np.zeros((C, H), np.float16)
    for h in range(H):
        blkd[h * DH:(h + 1) * DH, h] = 1.0
    ones1 = np.ones((1, C), np.float16)
    ident = np.eye(P, dtype=np.float16)

    # ---- per-type padded fp16 x ----
    def pad_rows(a, n):
        out = np.zeros((n, a.shape[1]), a.dtype)
        out[: a.shape[0]] = a
        return out

    xA16 = np.ascontiguousarray(pad_rows(xA.astype(np.float16), NPAD).T)  # [C, NPAD]
    xB16 = np.ascontiguousarray(pad_rows(xB.astype(np.float16), NPAD).T)

    # ---- edge prep ----
    meta = {"Rt": [], "visits": [], "tot16": [], "tot128": []}
    per_core = [dict() for _ in range(M)]
    rng_extra = 0
    for ri, (ekey, styp, dtyp) in enumerate(RELS):
        e = np.asarray(inputs[ekey])
        src = e[0].astype(np.int64)
        dst = e[1].astype(np.int64)
        core = dst // NT
        scid = (dst % NT) // SCN
        sub = src // SUBN
        key = (core * NSC + scid) * SUBT + sub
        order = np.lexsort((dst, key))
        src_s = src[order]
        dst_s = dst[order]
        key_s = key[order]
        counts = np.bincount(key_s, minlength=M * NSC * SUBT).reshape(M, NSC, SUBT)
        Rt = np.maximum(128, ((counts.max(axis=0) + 127) // 128) * 128)  # [NSC, SUBT]
        starts = np.zeros(M * NSC * SUBT + 1, np.int64)
        np.cumsum(counts.reshape(-1), out=starts[1:])

        # per-core arrays + per-batch chunk spans
        tot16 = int(Rt.sum() // 16)
        tot128 = int(Rt.sum() // 128)
        spans = {}  # (sc, sub, b) -> [cmin, cmax] union over cores
        for m in range(M):
            ik = np.zeros(int(Rt.sum()), np.int16)
            iq = np.zeros(int(Rt.sum()), np.int16)
            dr = np.full(int(Rt.sum()), -1.0, np.float16)
            off = 0
            for sc in range(NSC):
                for su in range(SUBT):
                    R = int(Rt[sc, su])
                    k = (m * NSC + sc) * SUBT + su
                    lo, hi = int(starts[k]), int(starts[k + 1])
                    n = hi - lo
                    ik[off:off + n] = (src_s[lo:hi] - su * SUBN).astype(np.int16)
                    iq[off:off + n] = (dst_s[lo:hi] - m * NT).astype(np.int16)
                    dl = (dst_s[lo:hi] - m * NT - sc * SCN).astype(np.int32)
                    dr[off:off + n] = dl.astype(np.float16)
                    for b in range(R // 128):
                        if b * 128 >= n:
                            break
                        c0 = int(dl[b * 128]) // 128
                        c1 = int(dl[min(b * 128 + 127, n - 1)]) // 128
                        kk = (sc, su, b)
                        if kk in spans:
                            spans[kk][0] = min(spans[kk][0], c0)
                            spans[kk][1] = max(spans[kk][1], c1)
                        else:
                            spans[kk] = [c0, c1]
                    off += R
            per_core[m][f"idxk{ri}"] = _wrap16(ik)
            per_core[m][f"idxq{ri}"] = _wrap16(iq)
            per_core[m][f"drel{ri}"] = _wrap128(dr)

        # build visit lists with per-BANK psum group start/stop flags (PSUM zero
        # regions are 2KB = one bank; only one accumulation group per bank, and
        # start zeroes the whole bank). One fused [msg|e8] matmul per visit:
        # chunk c -> bank c//3, offset (c%3)*136 (128 msg cols + 8 exp-sum cols).
        visits = []  # [sc][sub][b] -> list of (chunk, start, stop)
        for sc in range(NSC):
            order_v = []  # (sub, b, chunk) program order
            for su in range(SUBT):
                for b in range(int(Rt[sc, su]) // 128):
                    sp = spans.get((sc, su, b))
                    if sp is None:
                        continue
                    for c in range(sp[0], sp[1] + 1):
                        order_v.append((su, b, c))
            seen_banks = set(c // 3 for _, _, c in order_v)
            last_su = SUBT - 1
            last_b = int(Rt[sc, last_su]) // 128 - 1
            for bk in range(4):
                if bk not in seen_banks:
                    order_v.append((last_su, last_b, bk * 3))
            first = {}
            last = {}
            for j, (_, _, c) in enumerate(order_v):
                bk = c // 3
                if bk not in first:
                    first[bk] = j
                last[bk] = j
            vl = [[[] for _ in range(int(Rt[sc, su]) // 128)] for su in range(SUBT)]
            for j, (su, b, c) in enumerate(order_v):
                bk = c // 3
                vl[su][b].append((c, j == first[bk], j == last[bk]))
            visits.append(vl)

        meta["Rt"].append([[int(x) for x in row] for row in Rt])
        meta["visits"].append(visits)
        meta["tot16"].append(tot16)
        meta["tot128"].append(tot128)
        rng_extra += int(Rt.sum())

    meta["beta"] = [float(beta[0]), float(beta[1])]
    meta["zfa"] = bool(np.all(bfa == 0))
    meta["zfb"] = bool(np.all(bfb == 0))
    meta["zba"] = [bool(np.all(ba[t] == 0)) for t in (0, 1)]
    meta["zbqa"] = bool(np.all(bqa == 0))
    meta["zbqb"] = bool(np.all(bqb == 0))

    # ---- shared (replicated) inputs ----
    shared = {
        "xA16": xA16, "xB16": xB16,
        "wfa": wfa.astype(np.float16), "bfa": bfa.astype(np.float16)[None, :],
        "wfb": wfb.astype(np.float16), "bfb": bfb.astype(np.float16)[None, :],
        "wqa": wqa.astype(np.float16), "bqa": bqa.astype(np.float16)[None, :],
        "wqb": wqb.astype(np.float16), "bqb": bqb.astype(np.float16)[None, :],
        "waa": Wa[0].astype(np.float16), "baa": ba[0].astype(np.float16)[None, :],
        "wab": Wa[1].astype(np.float16), "bab": ba[1].astype(np.float16)[None, :],
        "iota": iota, "blkd": blkd, "ones1": ones1, "ident": ident,
    }
    for m in range(M):
        r0, r1 = m * NT, (m + 1) * NT
        per_core[m]["xad16"] = np.ascontiguousarray(pad_rows(xA[r0:r1].astype(np.float16), NTPAD).T)
        per_core[m]["xbd16"] = np.ascontiguousarray(pad_rows(xB[r0:r1].astype(np.float16), NTPAD).T)
        per_core[m]["xsa"] = np.ascontiguousarray((1.0 - beta[0]) * xA[r0:r1])
        per_core[m]["xsb"] = np.ascontiguousarray((1.0 - beta[1]) * xB[r0:r1])
        per_core[m].update(shared)
    return meta, per_core


def _build_nc(meta):
    import concourse.bacc as bacc
    import concourse.mybir as mybir
    import concourse.tile as tile

    f16 = mybir.dt.float16
    f32 = mybir.dt.float32
    i16 = mybir.dt.int16
    AF = mybir.ActivationFunctionType
    ALU = mybir.AluOpType

    nc = bacc.Bacc("TRN2", target_bir_lowering=False, debug=False, num_swdge_queues=4)

    # ---- I/O ----
    def din(name, shape, dt):
        return nc.dram_tensor(name, shape, dt, kind="ExternalInput")

    xA16 = din("xA16", [C, NPAD], f16)
    xB16 = din("xB16", [C, NPAD], f16)
    xad16 = din("xad16", [C, NTPAD], f16)
    xbd16 = din("xbd16", [C, NTPAD], f16)
    xsa = din("xsa", [NT, C], f32)
    xsb = din("xsb", [NT, C], f32)
    wfa = din("wfa", [C, 384], f16)
    bfa = din("bfa", [1, 384], f16)
    wfb = din("wfb", [C, 256], f16)
    bfb = din("bfb", [1, 256], f16)
    wqa = din("wqa", [C, 256], f16)
    bqa = din("bqa", [1, 256], f16)
    wqb = din("wqb", [C, 128], f16)
    bqb = din("bqb", [1, 128], f16)
    waa = din("waa", [C, C], f16)
    baa = din("baa", [1, C], f16)
    wab = din("wab", [C, C], f16)
    bab = din("bab", [1, C], f16)
    iota_d = din("iota", [P, NCH * 128], f16)
    blkd_d = din("blkd", [C, H], f16)
    ones1_d = din("ones1", [1, C], f16)
    ident_d = din("ident", [P, P], f16)
    idx_d = []
    for r in range(3):
        idx_d.append((
            din(f"idxk{r}", [P, meta["tot16"][r]], i16),
            din(f"idxq{r}", [P, meta["tot16"][r]], i16),
            din(f"drel{r}", [P, meta["tot128"][r]], f16),
        ))
    outA = nc.dram_tensor("outA", [NT, C], f32, kind="ExternalOutput")
    outB = nc.dram_tensor("outB", [NT, C], f32, kind="ExternalOutput")

    Rt = meta["Rt"]
    visits = meta["visits"]
    betaA, betaB = meta["beta"]

    with tile.TileContext(nc) as tc:
        with tc.tile_pool(name="dram", bufs=1, space="DRAM") as dram:
            fusedA = dram.tile([NPAD, 384], f16)
            fusedB = dram.tile([NPAD, 256], f16)
            qra = dram.tile([NTPAD, 256], f16)
            qrb = dram.tile([NTPAD, 128], f16)

            with tc.tile_pool(name="const", bufs=1) as cp:
                iota_sb = cp.tile([P, NCH * 128], f16)
                nc.sync.dma_start(iota_sb[:], iota_d[:])
                blkd_sb = cp.tile([C, H], f16)
                nc.sync.dma_start(blkd_sb[:], blkd_d[:])
                ones1_sb = cp.tile([1, C], f16)
                nc.sync.dma_start(ones1_sb[:], ones1_d[:])
                ident_sb = cp.tile([P, P], f16)
                nc.sync.dma_start(ident_sb[:], ident_d[:])
                w_sb = {}
                src_map = {"wfa": wfa, "bfa": bfa, "wfb": wfb, "bfb": bfb,
                           "wqa": wqa, "bqa": bqa, "wqb": wqb, "bqb": bqb,
                           "waa": waa, "baa": baa, "wab": wab, "bab": bab}
                for nm, dt_, sh in [("wfa", f16, [C, 384]), ("bfa", f16, [1, 384]),
                                    ("wfb", f16, [C, 256]), ("bfb", f16, [1, 256]),
                                    ("wqa", f16, [C, 256]), ("bqa", f16, [1, 256]),
                                    ("wqb", f16, [C, 128]), ("bqb", f16, [1, 128]),
                                    ("waa", f16, [C, C]), ("baa", f16, [1, C]),
                                    ("wab", f16, [C, C]), ("bab", f16, [1, C])]:
                    t = cp.tile(sh, dt_, tag=nm)
                    nc.sync.dma_start(t[:], src_map[nm][:])
                    w_sb[nm] = t

                import os as _osr
                _REP = int(_osr.environ.get("KERNEL_REPEAT", "1"))
                for _rep in range(_REP):
                    # ================= PHASE 1: build tables =================
                    with tc.tile_pool(name="prep", bufs=4) as pp, \
                         tc.tile_pool(name="prep_ps", bufs=3, space="PSUM") as pps:

                        def build_table(xdram, nrows, w, b, tbl, ncols, skip_bias):
                            GRP = 4  # node-tiles per DMA batch
                            ntiles = nrows // 128
                            base = 0
                            ii = 0
                            while base < ntiles:
                                grp = min(GRP, ntiles - base)
                                r0 = base * 128
                                xT = pp.tile([P, GRP, P], f16, tag="xT")
                                nc.sync.dma_start(
                                    xT[:, :grp, :], xdram[:, r0:r0 + grp * 128]
                                    .rearrange("c (t n) -> c t n", t=grp))
                                sb = pp.tile([P, GRP, ncols], f16, tag=f"sb{ncols}")
                                for t in range(grp):
                                    ps = pps.tile([P, 384], f32, tag="ps")
                                    nc.tensor.matmul(ps[:, :ncols], xT[:, t, :], w[:],
                                                     start=True, stop=skip_bias)
                                    if not skip_bias:
                                        nc.tensor.matmul(ps[:, :ncols], ones1_sb[:], b[:],
                                                         start=False, stop=True)
                                    if ii % 2 == 0:
                                        nc.scalar.copy(sb[:, t, :], ps[:, :ncols])
                                    else:
                                        nc.vector.tensor_copy(sb[:, t, :], ps[:, :ncols])
                                    ii += 1
                                nc.scalar.dma_start(
                                    tbl[r0:r0 + grp * 128, :]
                                    .rearrange("(t n) c -> n t c", n=128), sb[:, :grp, :])
                                base += grp

                        import os as _os1
                        if not _os1.environ.get("KERNEL_NOPREP"):
                            build_table(xA16, NPAD, w_sb["wfa"], w_sb["bfa"], fusedA, 384,
                                        meta["zfa"])
                            build_table(xB16, NPAD, w_sb["wfb"], w_sb["bfb"], fusedB, 256,
                                        meta["zfb"])
                            build_table(xad16, NTPAD, w_sb["wqa"], w_sb["bqa"], qra, 256,
                                        meta["zbqa"])
                            build_table(xbd16, NTPAD, w_sb["wqb"], w_sb["bqb"], qrb, 128,
                                        meta["zbqb"])

                    # ================= PHASE 2: streaming =================
                    with tc.tile_pool(name="agg", bufs=1) as apool:
                        agg = apool.tile([P, NSC * NCH, C], f16)

                        def out_stage(t):
                            import os as _os2
                            if _os2.environ.get("KERNEL_NOOUT"):
                                return
                            xs_d = xsa if t == 0 else xsb
                            out_d = outA if t == 0 else outB
                            wa = w_sb["waa" if t == 0 else "wab"]
                            bb = w_sb["baa" if t == 0 else "bab"]
                            bt = betaA if t == 0 else betaB
                            with tc.tile_pool(name="op", bufs=4) as op, \
                                 tc.tile_pool(name="ops", bufs=2, space="PSUM") as ops:
                                for slot in range(NSC * NCH):
                                    sc, ch = divmod(slot, NCH)
                                    rows = 98 if ch == 9 else 128
                                    base = sc * SCN + ch * 128
                                    g16 = op.tile([P, C], f16, tag="g16")
                                    nc.scalar.activation(g16[:], agg[:, slot, :], AF.Gelu)
                                    gt = ops.tile([P, C], f16, tag="gt")
                                    nc.tensor.transpose(gt[:], g16[:], ident_sb[:])
                                    gts = op.tile([P, C], f16, tag="gts")
                                    nc.vector.tensor_copy(gts[:], gt[:])
                                    o_ps = ops.tile([P, C], f32, tag="o")
                                    zb = meta["zba"][t]
                                    nc.tensor.matmul(o_ps[:], gts[:], wa[:], start=True, stop=zb)
                                    if not zb:
                                        nc.tensor.matmul(o_ps[:], ones1_sb[:], bb[:],
                                                         start=False, stop=True)
                                    xs = op.tile([P, C], f32, tag="xs")
                                    nc.sync.dma_start(xs[:rows, :], xs_d[base:base + rows, :])
                                    ob = op.tile([P, C], f32, tag="ob")
                                    nc.scalar.activation(ob[:], o_ps[:], AF.Copy, scale=float(bt))
                                    res = op.tile([P, C], f32, tag="res")
                                    nc.vector.tensor_add(res[:rows, :], ob[:rows, :], xs[:rows, :])
                                    nc.sync.dma_start(out_d[base:base + rows, :], res[:rows, :])

                        with tc.tile_pool(name="gidx", bufs=1) as gi, \
                             tc.tile_pool(name="gp", bufs=3) as gp, \
                             tc.tile_pool(name="ep", bufs=4) as ep:
                            import os as _os
                            n_rel = int(_os.environ.get("KERNEL_NREL", "3"))
                            gq = [0]  # round-robin swdge queue cursor

                            def nxq():
                                gq[0] = (gq[0] + 1) % 4
                                return gq[0]

                            for r, (ekey, styp, dtyp) in enumerate(RELS[:n_rel]):
                                ftab, fw = (fusedA, 384) if styp == 0 else (fusedB, 256)
                                vcol = 256 if r == 2 else 128
                                if r == 0:
                                    qtab, qw, qoff = qrb, 128, 0
                                elif r == 1:
                                    qtab, qw, qoff = qra, 256, 0
                                else:
                                    qtab, qw, qoff = qra, 256, 128
                                qap = qtab[:, qoff:qoff + 128]

                                idxk_sb = gi.tile([P, meta["tot16"][r]], i16, tag="idxk")
                                nc.sync.dma_start(idxk_sb[:], idx_d[r][0][:])
                                idxq_sb = gi.tile([P, meta["tot16"][r]], i16, tag="idxq")
                                nc.sync.dma_start(idxq_sb[:], idx_d[r][1][:])
                                drel_sb = gi.tile([P, meta["tot128"][r]], f16, tag="drel")
                                nc.sync.dma_start(drel_sb[:], idx_d[r][2][:])

                                with tc.tile_pool(name=f"agps{r}", bufs=2, space="PSUM") as agps:
                                    off16 = 0
                                    off128 = 0
                                    for sc in range(NSC):
                                        ag = agps.tile([P, 4, 512], f32, tag="aggps")
                                        for su in range(SUBT):
                                            R = Rt[r][sc][su]
                                            B = R // 128
                                            kap = ftab[su * SUBN:(su + 1) * SUBN, 0:128]
                                            vap = ftab[su * SUBN:(su + 1) * SUBN, vcol:vcol + 128]
                                            GC = 896  # per-gather idx cap (desc carveout is 1024)
                                            kg = gp.tile([P, B, 128], f16, tag="kg")
                                            qg = gp.tile([P, B, 128], f16, tag="qg")
                                            vg = gp.tile([P, B, 128], f16, tag="vg")
                                            for j0 in range(0, R, GC):
                                                n = min(GC, R - j0)
                                                i16s = idxk_sb[:, off16 + j0 // 16:off16 + (j0 + n) // 16]
                                                i16q = idxq_sb[:, off16 + j0 // 16:off16 + (j0 + n) // 16]
                                                nc.gpsimd.dma_gather(
                                                    kg[:, j0 // 128:(j0 + n) // 128, :], kap, i16s,
                                                    n, n, 128, elem_step=fw, transpose=False,
                                                    queue_num=nxq())
                                                nc.gpsimd.dma_gather(
                                                    qg[:, j0 // 128:(j0 + n) // 128, :], qap, i16q,
                                                    n, n, 128, elem_step=qw, transpose=False,
                                                    queue_num=nxq())
                                                nc.gpsimd.dma_gather(
                                                    vg[:, j0 // 128:(j0 + n) // 128, :], vap, i16s,
                                                    n, n, 128, elem_step=fw, transpose=False,
                                                    queue_num=nxq())
                                            prod = gp.tile([P, B, 128], f16, tag="prod")
                                            if not _os.environ.get("KERNEL_NOPROD"):
                                                nc.vector.tensor_mul(prod[:], kg[:], qg[:])
                                            lpr = gp.tile([P, B, H], f32, tag="lp")
                                            nc.vector.tensor_reduce(
                                                lpr[:], prod[:].rearrange("p b (h d) -> p b h d", d=DH),
                                                mybir.AxisListType.X, ALU.add)
                                            msg = gp.tile([P, B, 136], f16, tag="msg")
                                            nc.scalar.activation(msg[:, :, 128:136], lpr[:], AF.Exp)
                                            if not _os.environ.get("KERNEL_NOMSGMUL"):
                                                nc.vector.tensor_tensor(
                                                    out=msg[:, :, 0:128]
                                                    .rearrange("p b (h d) -> p b h d", d=DH),
                                                    in0=vg[:].rearrange("p b (h d) -> p b h d", d=DH),
                                                    in1=msg[:, :, 128:136]
                                                    .to_broadcast([P, B, H, DH]),
                                                    op=ALU.mult)
                                            for b in range(B):
                                                vlist = visits[r][sc][su][b]
                                                if vlist and not _os.environ.get("KERNEL_NOONEHOT"):
                                                    c0 = min(ch for ch, _, _ in vlist)
                                                    c1 = max(ch for ch, _, _ in vlist)
                                                    ohw = gp.tile([P, NCH * 128], f16, tag="ohw")
                                                    nc.vector.tensor_tensor(
                                                        out=ohw[:, c0 * 128:(c1 + 1) * 128],
                                                        in0=iota_sb[:, c0 * 128:(c1 + 1) * 128],
                                                        in1=drel_sb[:, off128 + b:off128 + b + 1]
                                                        .to_broadcast([P, (c1 + 1 - c0) * 128]),
                                                        op=ALU.is_equal)
                                                for (ch, ast, asp) in vlist:
                                                    oh_ap = (ohw[:, ch * 128:(ch + 1) * 128]
                                                             if not _os.environ.get("KERNEL_NOONEHOT")
                                                             else iota_sb[:, ch * 128:(ch + 1) * 128])
                                                    bk_, col = divmod(ch, 3)
                                                    nc.tensor.matmul(
                                                        ag[:, bk_, col * 136:col * 136 + 136],
                                                        oh_ap, msg[:, b, :], start=ast, stop=asp)
                                            off16 += R // 16
                                            off128 += B
                                        # epilogue for this superchunk
                                        for ch in range(NCH):
                                            bk_, col = divmod(ch, 3)
                                            a_ap = ag[:, bk_, col * 136:col * 136 + 128]
                                            s_ap = ag[:, bk_, col * 136 + 128:col * 136 + 136]
                                            rec = ep.tile([P, H], f32, tag="rec")
                                            nc.vector.tensor_scalar(rec[:], s_ap, 1e-16, None, op0=ALU.add)
                                            rec2 = ep.tile([P, H], f32, tag="rec2")
                                            nc.vector.reciprocal(rec2[:], rec[:])
                                            slot = sc * NCH + ch
                                            tgt = agg[:, slot, :].rearrange("p (h d) -> p h d", d=DH)
                                            src_v = a_ap.rearrange("p (h d) -> p h d", d=DH)
                                            if r == 2:
                                                tmp = ep.tile([P, C], f16, tag="tmp")
                                                nc.vector.tensor_tensor(
                                                    out=tmp[:].rearrange("p (h d) -> p h d", d=DH),
                                                    in0=src_v, in1=rec2[:].to_broadcast([P, H, DH]),
                                                    op=ALU.mult)
                                                nc.vector.tensor_add(agg[:, slot, :], agg[:, slot, :], tmp[:])
                                            else:
                                                nc.vector.tensor_tensor(
                                                    out=tgt, in0=src_v,
                                                    in1=rec2[:].to_broadcast([P, H, DH]),
                                                    op=ALU.mult)
                                if r == 0:
                                    out_stage(1)
                            out_stage(0)
    nc.compile()
    return nc


def _meta_key(meta):
    import json
    return json.dumps(meta, sort_keys=True)


def kernel(**inputs):
    meta, per_core = _host_prep(inputs)
    key = _meta_key(meta)
    if key not in _CACHE:
        _CACHE.clear()
        _CACHE[key] = _build_nc(meta)
    nc = _CACHE[key]

    from concourse.bass_utils import run_bass_kernel_spmd
    import os
    trace = bool(int(os.environ.get("KERNEL_TRACE", "0")))
    res = run_bass_kernel_spmd(nc, per_core, core_ids=list(range(M)), trace=trace)
    if trace:
        kernel.last_exec_time_ns = res.exec_time_ns
        kernel.last_trace = res.instructions_and_trace
    outs = res.results
    outA = np.concatenate([outs[m]["outA"] for m in range(M)], axis=0)
    outB = np.concatenate([outs[m]["outB"] for m in range(M)], axis=0)
    return np.stack([outA, outB]).astype(np.float32)



# revision 50
# speedup vs baseline: 2.1218x; 1.2145x over previous
"""Trainium2 Bass kernel for DragonHGT (heterogeneous graph transformer layer).

Strategy (8 NeuronCores, no collectives):
  - Shard edges by DESTINATION node range: core i owns dst nodes [i*12500, (i+1)*12500)
    of both node types. All segment ops (softmax denom, aggregation) become core-local.
  - Fold per-relation transforms into host-fused weights:
      qr = q @ a_rel^T * scale * p_rel   (folded into Wqr per relation, dst-side)
      vr = v @ m_rel                     (folded into Wvr per relation, src-side)
    so logits = <qr[dst], k[src]> per head and messages need no per-edge small matmuls.
  - Skip segment-max (logits are O(6) here; exp is safe in fp32/fp16 range) and
    normalize AFTER aggregation: agg = (sum_e e_e * vr_src) / (sum_e e_e).
  - Host bucket-sorts edges by (core, superchunk-of-1250-dst, src-subtable-of-25k, dst)
    so that src gathers use int16 dma_gather (fast SWDGE path) and the segment-sum
    is a one-hot matmul into PSUM per 128-node chunk.
  - Tables (k | vr...) are built on-device (replicated across cores) as fp16 HBM
    tables, then gathered per edge with dma_gather (k,qr transposed; vr plain).
"""

import math

import numpy as np

P = 128
NN = 100000          # nodes per type
C = 128
H = 8
DH = 16
M = 8                # cores
NT = NN // M         # 12500 dst rows per core
SCN = 1250           # dst nodes per superchunk
NSC = NT // SCN      # 10 superchunks per core
NCH = 10             # 128-node chunks per superchunk (9*128 + 98)
SUBT = 4             # src subtables
SUBN = NN // SUBT    # 25000
NPAD = 782 * 128     # 100096 (full tables padded)
NTPAD = 98 * 128     # 12544  (dst tables padded)
SCALE = 1.0 / math.sqrt(DH)

# relations: (edge_key, src_type, dst_type)
RELS = [("eAB", 0, 1), ("eBA", 1, 0), ("eAA", 0, 0)]

_CACHE = {}


def _sigmoid(x):
    return 1.0 / (1.0 + np.exp(-x))


def _blockdiag(mats):
    """mats: [H, DH, DH] -> [C, C] block diagonal."""
    out = np.zeros((C, C), np.float32)
    for h in range(H):
        out[h * DH:(h + 1) * DH, h * DH:(h + 1) * DH] = mats[h]
    return out


def _wrap16(arr_i16):
    """[R] int16 -> [128, R//16] wrapped (idx j at [j%16, j//16]) replicated to 128 partitions."""
    R = arr_i16.shape[0]
    w = arr_i16.reshape(R // 16, 16).T  # [16, R/16]
    return np.tile(w, (8, 1))


def _wrap128(arr):
    """[R] -> [128, R//128] (edge j at [j%128, j//128])."""
    R = arr.shape[0]
    return np.ascontiguousarray(arr.reshape(R // 128, 128).T)


def _host_prep(inputs):
    """Returns (meta, per_core_inputs). meta is SPMD-identical; arrays differ per core."""
    xA = np.asarray(inputs["xA"], np.float32)
    xB = np.asarray(inputs["xB"], np.float32)
    Wk = np.asarray(inputs["Wk"], np.float32)
    bk = np.asarray(inputs["bk"], np.float32)
    Wq = np.asarray(inputs["Wq"], np.float32)
    bq = np.asarray(inputs["bq"], np.float32)
    Wv = np.asarray(inputs["Wv"], np.float32)
    bv = np.asarray(inputs["bv"], np.float32)
    Wa = np.asarray(inputs["Wa"], np.float32)
    ba = np.asarray(inputs["ba"], np.float32)
    skip = np.asarray(inputs["skip"], np.float32)
    a_rel = np.asarray(inputs["a_rel"], np.float32)
    m_rel = np.asarray(inputs["m_rel"], np.float32)
    p_rel = np.asarray(inputs["p_rel"], np.float32)

    beta = _sigmoid(skip)  # [2]

    # ---- fused weights ----
    # A-type src table: [ kA | vr(rel0) | kA | vr(rel2) ] so each relation's
    # k|v pair is one contiguous 256-element gather row.
    blkM = [_blockdiag(m_rel[r]) for r in range(3)]
    wfa = np.concatenate([Wk[0], Wv[0] @ blkM[0], Wk[0], Wv[0] @ blkM[2]], axis=1)  # [128,512]
    bfa = np.concatenate([bk[0], bv[0] @ blkM[0], bk[0], bv[0] @ blkM[2]])          # [512]
    wfb = np.concatenate([Wk[1], Wv[1] @ blkM[1]], axis=1)                          # [128,256]
    bfb = np.concatenate([bk[1], bv[1] @ blkM[1]])
    # qr weights: qr_r = q_t(r) @ blkdiag(a_rel[r].T) * scale * p_rel[r,h]
    blkQ = []
    for r in range(3):
        mats = [a_rel[r, h].T * (SCALE * p_rel[r, h]) for h in range(H)]
        blkQ.append(_blockdiag(np.stack(mats)))
    # dst types: rel0 -> B, rel1 -> A, rel2 -> A
    wqb = Wq[1] @ blkQ[0]
    bqb = bq[1] @ blkQ[0]
    wqa = np.concatenate([Wq[0] @ blkQ[1], Wq[0] @ blkQ[2]], axis=1)  # [128,256]
    bqa = np.concatenate([bq[0] @ blkQ[1], bq[0] @ blkQ[2]])

    # ---- consts ----
    iota = np.tile(np.arange(SCN + 30, dtype=np.float32)[None, :NCH * 128], (P, 1)).astype(np.float16)
    blkd = np.zeros((C, H), np.float16)
    for h in range(H):
        blkd[h * DH:(h + 1) * DH, h] = 1.0
    ones1 = np.ones((1, C), np.float16)
    ident = np.eye(P, dtype=np.float16)

    # ---- per-type padded fp16 x ----
    def pad_rows(a, n):
        out = np.zeros((n, a.shape[1]), a.dtype)
        out[: a.shape[0]] = a
        return out

    xA16 = np.ascontiguousarray(pad_rows(xA.astype(np.float16), NPAD).T)  # [C, NPAD]
    xB16 = np.ascontiguousarray(pad_rows(xB.astype(np.float16), NPAD).T)

    # ---- edge prep ----
    meta = {"Rt": [], "visits": [], "tot16": [], "tot128": []}
    per_core = [dict() for _ in range(M)]
    rng_extra = 0
    for ri, (ekey, styp, dtyp) in enumerate(RELS):
        e = np.asarray(inputs[ekey])
        src = e[0].astype(np.int64)
        dst = e[1].astype(np.int64)
        core = dst // NT
        scid = (dst % NT) // SCN
        sub = src // SUBN
        key = (core * NSC + scid) * SUBT + sub
        order = np.lexsort((dst, key))
        src_s = src[order]
        dst_s = dst[order]
        key_s = key[order]
        counts = np.bincount(key_s, minlength=M * NSC * SUBT).reshape(M, NSC, SUBT)
        Rt = np.maximum(128, ((counts.max(axis=0) + 127) // 128) * 128)  # [NSC, SUBT]
        starts = np.zeros(M * NSC * SUBT + 1, np.int64)
        np.cumsum(counts.reshape(-1), out=starts[1:])

        # per-core arrays + per-batch chunk spans
        tot16 = int(Rt.sum() // 16)
        tot128 = int(Rt.sum() // 128)
        spans = {}  # (sc, sub, b) -> [cmin, cmax] union over cores
        for m in range(M):
            ik = np.zeros(int(Rt.sum()), np.int16)
            iq = np.zeros(int(Rt.sum()), np.int16)
            dr = np.full(int(Rt.sum()), -1.0, np.float16)
            off = 0
            for sc in range(NSC):
                for su in range(SUBT):
                    R = int(Rt[sc, su])
                    k = (m * NSC + sc) * SUBT + su
                    lo, hi = int(starts[k]), int(starts[k + 1])
                    n = hi - lo
                    ik[off:off + n] = (src_s[lo:hi] - su * SUBN).astype(np.int16)
                    iq[off:off + n] = (dst_s[lo:hi] - m * NT).astype(np.int16)
                    dl = (dst_s[lo:hi] - m * NT - sc * SCN).astype(np.int32)
                    dr[off:off + n] = dl.astype(np.float16)
                    for b in range(R // 128):
                        if b * 128 >= n:
                            break
                        c0 = int(dl[b * 128]) // 128
                        c1 = int(dl[min(b * 128 + 127, n - 1)]) // 128
                        kk = (sc, su, b)
                        if kk in spans:
                            spans[kk][0] = min(spans[kk][0], c0)
                            spans[kk][1] = max(spans[kk][1], c1)
                        else:
                            spans[kk] = [c0, c1]
                    off += R
            per_core[m][f"idxk{ri}"] = _wrap16(ik)
            per_core[m][f"idxq{ri}"] = _wrap16(iq)
            per_core[m][f"drel{ri}"] = _wrap128(dr)
            per_core[m][f"drln{ri}"] = _wrap128((-dr).astype(np.float32))

        # build visit lists with per-BANK psum group start/stop flags (PSUM zero
        # regions are 2KB = one bank; only one accumulation group per bank, and
        # start zeroes the whole bank). One fused [msg|e8] matmul per visit:
        # chunk c -> bank c//3, offset (c%3)*136 (128 msg cols + 8 exp-sum cols).
        visits = []  # [sc][sub][b] -> list of (chunk, start, stop)
        for sc in range(NSC):
            order_v = []  # (sub, b, chunk) program order
            for su in range(SUBT):
                for b in range(int(Rt[sc, su]) // 128):
                    sp = spans.get((sc, su, b))
                    if sp is None:
                        continue
                    for c in range(sp[0], sp[1] + 1):
                        order_v.append((su, b, c))
            seen_banks = set(c // 3 for _, _, c in order_v)
            last_su = SUBT - 1
            last_b = int(Rt[sc, last_su]) // 128 - 1
            for bk in range(4):
                if bk not in seen_banks:
                    order_v.append((last_su, last_b, bk * 3))
            first = {}
            last = {}
            for j, (_, _, c) in enumerate(order_v):
                bk = c // 3
                if bk not in first:
                    first[bk] = j
                last[bk] = j
            vl = [[[] for _ in range(int(Rt[sc, su]) // 128)] for su in range(SUBT)]
            for j, (su, b, c) in enumerate(order_v):
                bk = c // 3
                vl[su][b].append((c, j == first[bk], j == last[bk]))
            visits.append(vl)

        meta["Rt"].append([[int(x) for x in row] for row in Rt])
        meta["visits"].append(visits)
        meta["tot16"].append(tot16)
        meta["tot128"].append(tot128)
        rng_extra += int(Rt.sum())

    meta["beta"] = [float(beta[0]), float(beta[1])]
    meta["zfa"] = bool(np.all(bfa == 0))
    meta["zfb"] = bool(np.all(bfb == 0))
    meta["zba"] = [bool(np.all(ba[t] == 0)) for t in (0, 1)]
    meta["zbqa"] = bool(np.all(bqa == 0))
    meta["zbqb"] = bool(np.all(bqb == 0))

    # ---- shared (replicated) inputs ----
    shared = {
        "xA16": xA16, "xB16": xB16,
        "wfa": wfa.astype(np.float16), "bfa": bfa.astype(np.float16)[None, :],
        "wfb": wfb.astype(np.float16), "bfb": bfb.astype(np.float16)[None, :],
        "wqa": wqa.astype(np.float16), "bqa": bqa.astype(np.float16)[None, :],
        "wqb": wqb.astype(np.float16), "bqb": bqb.astype(np.float16)[None, :],
        "waa": Wa[0].astype(np.float16), "baa": ba[0].astype(np.float16)[None, :],
        "wab": Wa[1].astype(np.float16), "bab": ba[1].astype(np.float16)[None, :],
        "iota": iota, "blkd": blkd, "ones1": ones1, "ident": ident,
    }
    for m in range(M):
        r0, r1 = m * NT, (m + 1) * NT
        per_core[m]["xad16"] = np.ascontiguousarray(pad_rows(xA[r0:r1].astype(np.float16), NTPAD).T)
        per_core[m]["xbd16"] = np.ascontiguousarray(pad_rows(xB[r0:r1].astype(np.float16), NTPAD).T)
        per_core[m]["xsa"] = np.ascontiguousarray((1.0 - beta[0]) * xA[r0:r1])
        per_core[m]["xsb"] = np.ascontiguousarray((1.0 - beta[1]) * xB[r0:r1])
        per_core[m].update(shared)
    return meta, per_core


def _build_nc(meta):
    import concourse.bacc as bacc
    import concourse.mybir as mybir
    import concourse.tile as tile

    f16 = mybir.dt.float16
    f32 = mybir.dt.float32
    i16 = mybir.dt.int16
    AF = mybir.ActivationFunctionType
    ALU = mybir.AluOpType

    nc = bacc.Bacc("TRN2", target_bir_lowering=False, debug=False, num_swdge_queues=4)

    # ---- I/O ----
    def din(name, shape, dt):
        return nc.dram_tensor(name, shape, dt, kind="ExternalInput")

    xA16 = din("xA16", [C, NPAD], f16)
    xB16 = din("xB16", [C, NPAD], f16)
    xad16 = din("xad16", [C, NTPAD], f16)
    xbd16 = din("xbd16", [C, NTPAD], f16)
    xsa = din("xsa", [NT, C], f32)
    xsb = din("xsb", [NT, C], f32)
    wfa = din("wfa", [C, 512], f16)
    bfa = din("bfa", [1, 512], f16)
    wfb = din("wfb", [C, 256], f16)
    bfb = din("bfb", [1, 256], f16)
    wqa = din("wqa", [C, 256], f16)
    bqa = din("bqa", [1, 256], f16)
    wqb = din("wqb", [C, 128], f16)
    bqb = din("bqb", [1, 128], f16)
    waa = din("waa", [C, C], f16)
    baa = din("baa", [1, C], f16)
    wab = din("wab", [C, C], f16)
    bab = din("bab", [1, C], f16)
    iota_d = din("iota", [P, NCH * 128], f16)
    blkd_d = din("blkd", [C, H], f16)
    ones1_d = din("ones1", [1, C], f16)
    ident_d = din("ident", [P, P], f16)
    idx_d = []
    for r in range(3):
        idx_d.append((
            din(f"idxk{r}", [P, meta["tot16"][r]], i16),
            din(f"idxq{r}", [P, meta["tot16"][r]], i16),
            din(f"drel{r}", [P, meta["tot128"][r]], f16),
            din(f"drln{r}", [P, meta["tot128"][r]], f32),
        ))
    outA = nc.dram_tensor("outA", [NT, C], f32, kind="ExternalOutput")
    outB = nc.dram_tensor("outB", [NT, C], f32, kind="ExternalOutput")

    Rt = meta["Rt"]
    visits = meta["visits"]
    betaA, betaB = meta["beta"]

    with tile.TileContext(nc) as tc:
        with tc.tile_pool(name="dram", bufs=1, space="DRAM") as dram:
            fusedA = dram.tile([NPAD, 512], f16)
            fusedB = dram.tile([NPAD, 256], f16)
            qra = dram.tile([NTPAD, 256], f16)
            qrb = dram.tile([NTPAD, 128], f16)

            with tc.tile_pool(name="const", bufs=1) as cp:
                iota_sb = cp.tile([P, NCH * 128], f16)
                nc.sync.dma_start(iota_sb[:], iota_d[:])
                blkd_sb = cp.tile([C, H], f16)
                nc.sync.dma_start(blkd_sb[:], blkd_d[:])
                ones1_sb = cp.tile([1, C], f16)
                nc.sync.dma_start(ones1_sb[:], ones1_d[:])
                ident_sb = cp.tile([P, P], f16)
                nc.sync.dma_start(ident_sb[:], ident_d[:])
                w_sb = {}
                src_map = {"wfa": wfa, "bfa": bfa, "wfb": wfb, "bfb": bfb,
                           "wqa": wqa, "bqa": bqa, "wqb": wqb, "bqb": bqb,
                           "waa": waa, "baa": baa, "wab": wab, "bab": bab}
                for nm, dt_, sh in [("wfa", f16, [C, 512]), ("bfa", f16, [1, 512]),
                                    ("wfb", f16, [C, 256]), ("bfb", f16, [1, 256]),
                                    ("wqa", f16, [C, 256]), ("bqa", f16, [1, 256]),
                                    ("wqb", f16, [C, 128]), ("bqb", f16, [1, 128]),
                                    ("waa", f16, [C, C]), ("baa", f16, [1, C]),
                                    ("wab", f16, [C, C]), ("bab", f16, [1, C])]:
                    t = cp.tile(sh, dt_, tag=nm)
                    nc.sync.dma_start(t[:], src_map[nm][:])
                    w_sb[nm] = t

                import os as _osr
                _REP = int(_osr.environ.get("KERNEL_REPEAT", "1"))
                for _rep in range(_REP):
                    # ================= PHASE 1: build tables =================
                    with tc.tile_pool(name="prep", bufs=4) as pp, \
                         tc.tile_pool(name="prep_ps", bufs=3, space="PSUM") as pps:

                        def build_table(xdram, nrows, w, b, tbl, ncols, skip_bias):
                            GRP = 4  # node-tiles per DMA batch
                            ntiles = nrows // 128
                            base = 0
                            ii = 0
                            while base < ntiles:
                                grp = min(GRP, ntiles - base)
                                r0 = base * 128
                                xT = pp.tile([P, GRP, P], f16, tag="xT")
                                nc.sync.dma_start(
                                    xT[:, :grp, :], xdram[:, r0:r0 + grp * 128]
                                    .rearrange("c (t n) -> c t n", t=grp))
                                sb = pp.tile([P, GRP, ncols], f16, tag=f"sb{ncols}")
                                for t in range(grp):
                                    ps = pps.tile([P, 512], f32, tag="ps")
                                    nc.tensor.matmul(ps[:, :ncols], xT[:, t, :], w[:],
                                                     start=True, stop=skip_bias)
                                    if not skip_bias:
                                        nc.tensor.matmul(ps[:, :ncols], ones1_sb[:], b[:],
                                                         start=False, stop=True)
                                    nc.scalar.copy(sb[:, t, :], ps[:, :ncols])
                                    ii += 1
                                nc.scalar.dma_start(
                                    tbl[r0:r0 + grp * 128, :]
                                    .rearrange("(t n) c -> n t c", n=128), sb[:, :grp, :])
                                base += grp

                        import os as _os1
                        if not _os1.environ.get("KERNEL_NOPREP"):
                            build_table(xA16, NPAD, w_sb["wfa"], w_sb["bfa"], fusedA, 512,
                                        meta["zfa"])
                            build_table(xB16, NPAD, w_sb["wfb"], w_sb["bfb"], fusedB, 256,
                                        meta["zfb"])
                            build_table(xad16, NTPAD, w_sb["wqa"], w_sb["bqa"], qra, 256,
                                        meta["zbqa"])
                            build_table(xbd16, NTPAD, w_sb["wqb"], w_sb["bqb"], qrb, 128,
                                        meta["zbqb"])

                    # ================= PHASE 2: streaming =================
                    with tc.tile_pool(name="agg", bufs=1) as apool:
                        agg = apool.tile([P, NSC * NCH, C], f16)

                        def out_stage(t):
                            import os as _os2
                            if _os2.environ.get("KERNEL_NOOUT"):
                                return
                            xs_d = xsa if t == 0 else xsb
                            out_d = outA if t == 0 else outB
                            wa = w_sb["waa" if t == 0 else "wab"]
                            bb = w_sb["baa" if t == 0 else "bab"]
                            bt = betaA if t == 0 else betaB
                            with tc.tile_pool(name="op", bufs=4) as op, \
                                 tc.tile_pool(name="ops", bufs=2, space="PSUM") as ops:
                                for slot in range(NSC * NCH):
                                    sc, ch = divmod(slot, NCH)
                                    rows = 98 if ch == 9 else 128
                                    base = sc * SCN + ch * 128
                                    g16 = op.tile([P, C], f16, tag="g16")
                                    nc.scalar.activation(g16[:], agg[:, slot, :], AF.Gelu)
                                    gt = ops.tile([P, C], f16, tag="gt")
                                    nc.tensor.transpose(gt[:], g16[:], ident_sb[:])
                                    gts = op.tile([P, C], f16, tag="gts")
                                    nc.vector.tensor_copy(gts[:], gt[:])
                                    o_ps = ops.tile([P, C], f32, tag="o")
                                    zb = meta["zba"][t]
                                    nc.tensor.matmul(o_ps[:], gts[:], wa[:], start=True, stop=zb)
                                    if not zb:
                                        nc.tensor.matmul(o_ps[:], ones1_sb[:], bb[:],
                                                         start=False, stop=True)
                                    xs = op.tile([P, C], f32, tag="xs")
                                    nc.sync.dma_start(xs[:rows, :], xs_d[base:base + rows, :])
                                    ob = op.tile([P, C], f32, tag="ob")
                                    nc.scalar.activation(ob[:], o_ps[:], AF.Copy, scale=float(bt))
                                    res = op.tile([P, C], f32, tag="res")
                                    nc.vector.tensor_add(res[:rows, :], ob[:rows, :], xs[:rows, :])
                                    nc.sync.dma_start(out_d[base:base + rows, :], res[:rows, :])

                        with tc.tile_pool(name="gidx", bufs=1) as gi, \
                             tc.tile_pool(name="gp", bufs=3) as gp, \
                             tc.tile_pool(name="ep", bufs=4) as ep:
                            import os as _os
                            n_rel = int(_os.environ.get("KERNEL_NREL", "3"))
                            obc = [0]  # one-hot engine split counter
                            gq = [0]  # round-robin swdge queue cursor

                            def nxq():
                                gq[0] = (gq[0] + 1) % 4
                                return gq[0]

                            for r, (ekey, styp, dtyp) in enumerate(RELS[:n_rel]):
                                ftab, fw = (fusedA, 512) if styp == 0 else (fusedB, 256)
                                kvcol = 256 if r == 2 else 0
                                if r == 0:
                                    qtab, qw, qoff = qrb, 128, 0
                                elif r == 1:
                                    qtab, qw, qoff = qra, 256, 0
                                else:
                                    qtab, qw, qoff = qra, 256, 128
                                qap = qtab[:, qoff:qoff + 128]

                                idxk_sb = gi.tile([P, meta["tot16"][r]], i16, tag="idxk")
                                nc.sync.dma_start(idxk_sb[:], idx_d[r][0][:])
                                idxq_sb = gi.tile([P, meta["tot16"][r]], i16, tag="idxq")
                                nc.sync.dma_start(idxq_sb[:], idx_d[r][1][:])
                                drel_sb = gi.tile([P, meta["tot128"][r]], f16, tag="drel")
                                nc.sync.dma_start(drel_sb[:], idx_d[r][2][:])
                                drln_sb = gi.tile([P, meta["tot128"][r]], f32, tag="drln")
                                nc.sync.dma_start(drln_sb[:], idx_d[r][3][:])

                                with tc.tile_pool(name=f"agps{r}", bufs=2, space="PSUM") as agps:
                                    off16 = 0
                                    off128 = 0
                                    for sc in range(NSC):
                                        ag = agps.tile([P, 4, 512], f32, tag="aggps")
                                        for su in range(SUBT):
                                            R = Rt[r][sc][su]
                                            B = R // 128
                                            kvap = ftab[su * SUBN:(su + 1) * SUBN, kvcol:kvcol + 256]
                                            GC = 896  # per-gather idx cap (desc carveout is 1024)
                                            kv = gp.tile([P, B, 256], f16, tag="kv")
                                            qg = gp.tile([P, B, 128], f16, tag="qg")
                                            for j0 in range(0, R, GC):
                                                n = min(GC, R - j0)
                                                i16s = idxk_sb[:, off16 + j0 // 16:off16 + (j0 + n) // 16]
                                                i16q = idxq_sb[:, off16 + j0 // 16:off16 + (j0 + n) // 16]
                                                nc.gpsimd.dma_gather(
                                                    kv[:, j0 // 128:(j0 + n) // 128, :], kvap, i16s,
                                                    n, n, 256, elem_step=fw, transpose=False,
                                                    queue_num=nxq())
                                                nc.gpsimd.dma_gather(
                                                    qg[:, j0 // 128:(j0 + n) // 128, :], qap, i16q,
                                                    n, n, 128, elem_step=qw, transpose=False,
                                                    queue_num=nxq())
                                            prod = gp.tile([P, B, 128], f16, tag="prod")
                                            if not _os.environ.get("KERNEL_NOPROD"):
                                                nc.vector.tensor_mul(prod[:], kv[:, :, 0:128], qg[:])
                                            lpr = gp.tile([P, B, H], f16, tag="lp")
                                            with nc.allow_low_precision("16-term head reduce; logits O(6)"):
                                                nc.vector.tensor_reduce(
                                                    lpr[:], prod[:].rearrange("p b (h d) -> p b h d", d=DH),
                                                    mybir.AxisListType.X, ALU.add)
                                            msg = gp.tile([P, B, 136], f16, tag="msg")
                                            nc.scalar.activation(msg[:, :, 128:136], lpr[:], AF.Exp)
                                            if not _os.environ.get("KERNEL_NOMSGMUL"):
                                                nc.vector.tensor_tensor(
                                                    out=msg[:, :, 0:128]
                                                    .rearrange("p b (h d) -> p b h d", d=DH),
                                                    in0=kv[:, :, 128:256]
                                                    .rearrange("p b (h d) -> p b h d", d=DH),
                                                    in1=msg[:, :, 128:136]
                                                    .to_broadcast([P, B, H, DH]),
                                                    op=ALU.mult)
                                            for b in range(B):
                                                vlist = visits[r][sc][su][b]
                                                if vlist and not _os.environ.get("KERNEL_NOONEHOT"):
                                                    c0 = min(ch for ch, _, _ in vlist)
                                                    c1 = max(ch for ch, _, _ in vlist)
                                                    w0, w1 = c0 * 128, (c1 + 1) * 128
                                                    ohw = gp.tile([P, NCH * 128], f16, tag="ohw")
                                                    obc[0] += 1
                                                    if obc[0] % 3 == 2:
                                                        oht = gp.tile([P, NCH * 128], f16, tag="oht")
                                                        nc.scalar.activation(
                                                            oht[:, w0:w1], iota_sb[:, w0:w1], AF.Abs,
                                                            bias=drln_sb[:, off128 + b:off128 + b + 1])
                                                        nc.scalar.activation(
                                                            ohw[:, w0:w1], oht[:, w0:w1], AF.Relu,
                                                            bias=1.0, scale=-1.0)
                                                    else:
                                                        nc.vector.tensor_tensor(
                                                            out=ohw[:, w0:w1],
                                                            in0=iota_sb[:, w0:w1],
                                                            in1=drel_sb[:, off128 + b:off128 + b + 1]
                                                            .to_broadcast([P, w1 - w0]),
                                                            op=ALU.is_equal)
                                                for (ch, ast, asp) in vlist:
                                                    oh_ap = (ohw[:, ch * 128:(ch + 1) * 128]
                                                             if not _os.environ.get("KERNEL_NOONEHOT")
                                                             else iota_sb[:, ch * 128:(ch + 1) * 128])
                                                    bk_, col = divmod(ch, 3)
                                                    nc.tensor.matmul(
                                                        ag[:, bk_, col * 136:col * 136 + 136],
                                                        oh_ap, msg[:, b, :], start=ast, stop=asp)
                                            off16 += R // 16
                                            off128 += B
                                        # epilogue for this superchunk
                                        for ch in range(NCH):
                                            bk_, col = divmod(ch, 3)
                                            a_ap = ag[:, bk_, col * 136:col * 136 + 128]
                                            s_ap = ag[:, bk_, col * 136 + 128:col * 136 + 136]
                                            rec = ep.tile([P, H], f32, tag="rec")
                                            nc.vector.tensor_scalar(rec[:], s_ap, 1e-16, None, op0=ALU.add)
                                            rec2 = ep.tile([P, H], f32, tag="rec2")
                                            nc.vector.reciprocal(rec2[:], rec[:])
                                            slot = sc * NCH + ch
                                            tgt = agg[:, slot, :].rearrange("p (h d) -> p h d", d=DH)
                                            src_v = a_ap.rearrange("p (h d) -> p h d", d=DH)
                                            if r == 2:
                                                tmp = ep.tile([P, C], f16, tag="tmp")
                                                nc.vector.tensor_tensor(
                                                    out=tmp[:].rearrange("p (h d) -> p h d", d=DH),
                                                    in0=src_v, in1=rec2[:].to_broadcast([P, H, DH]),
                                                    op=ALU.mult)
                                                nc.vector.tensor_add(agg[:, slot, :], agg[:, slot, :], tmp[:])
                                            else:
                                                nc.vector.tensor_tensor(
                                                    out=tgt, in0=src_v,
                                                    in1=rec2[:].to_broadcast([P, H, DH]),
                                                    op=ALU.mult)
                                if r == 0:
                                    out_stage(1)
                            out_stage(0)
    nc.compile()
    return nc


def _meta_key(meta):
    import json
    return json.dumps(meta, sort_keys=True)


def kernel(**inputs):
    meta, per_core = _host_prep(inputs)
    key = _meta_key(meta)
    if key not in _CACHE:
        _CACHE.clear()
        _CACHE[key] = _build_nc(meta)
    nc = _CACHE[key]

    from concourse.bass_utils import run_bass_kernel_spmd
    import os
    trace = bool(int(os.environ.get("KERNEL_TRACE", "0")))
    res = run_bass_kernel_spmd(nc, per_core, core_ids=list(range(M)), trace=trace)
    if trace:
        kernel.last_exec_time_ns = res.exec_time_ns
        kernel.last_trace = res.instructions_and_trace
    outs = res.results
    outA = np.concatenate([outs[m]["outA"] for m in range(M)], axis=0)
    outB = np.concatenate([outs[m]["outB"] for m in range(M)], axis=0)
    return np.stack([outA, outB]).astype(np.float32)



# revision 53
# speedup vs baseline: 2.2024x; 1.0380x over previous
"""Trainium2 Bass kernel for DragonHGT (heterogeneous graph transformer layer).

Strategy (8 NeuronCores, no collectives):
  - Shard edges by DESTINATION node range: core i owns dst nodes [i*12500, (i+1)*12500)
    of both node types. All segment ops (softmax denom, aggregation) become core-local.
  - Fold per-relation transforms into host-fused weights:
      qr = q @ a_rel^T * scale * p_rel   (folded into Wqr per relation, dst-side)
      vr = v @ m_rel                     (folded into Wvr per relation, src-side)
    so logits = <qr[dst], k[src]> per head and messages need no per-edge small matmuls.
  - Skip segment-max (logits are O(6) here; exp is safe in fp32/fp16 range) and
    normalize AFTER aggregation: agg = (sum_e e_e * vr_src) / (sum_e e_e).
  - Host bucket-sorts edges by (core, superchunk-of-1250-dst, src-subtable-of-25k, dst)
    so that src gathers use int16 dma_gather (fast SWDGE path) and the segment-sum
    is a one-hot matmul into PSUM per 128-node chunk.
  - Tables (k | vr...) are built on-device (replicated across cores) as fp16 HBM
    tables, then gathered per edge with dma_gather (k,qr transposed; vr plain).
"""

import math

import numpy as np

P = 128
NN = 100000          # nodes per type
C = 128
H = 8
DH = 16
M = 8                # cores
NT = NN // M         # 12500 dst rows per core
SCN = 1250           # dst nodes per superchunk
NSC = NT // SCN      # 10 superchunks per core
NCH = 10             # 128-node chunks per superchunk (9*128 + 98)
SUBT = 4             # src subtables
SUBN = NN // SUBT    # 25000
NPAD = 782 * 128     # 100096 (full tables padded)
NTPAD = 98 * 128     # 12544  (dst tables padded)
SCALE = 1.0 / math.sqrt(DH)

# relations: (edge_key, src_type, dst_type)
RELS = [("eAB", 0, 1), ("eBA", 1, 0), ("eAA", 0, 0)]

_CACHE = {}


def _sigmoid(x):
    return 1.0 / (1.0 + np.exp(-x))


def _blockdiag(mats):
    """mats: [H, DH, DH] -> [C, C] block diagonal."""
    out = np.zeros((C, C), np.float32)
    for h in range(H):
        out[h * DH:(h + 1) * DH, h * DH:(h + 1) * DH] = mats[h]
    return out


def _wrap16(arr_i16):
    """[R] int16 -> [128, R//16] wrapped (idx j at [j%16, j//16]) replicated to 128 partitions."""
    R = arr_i16.shape[0]
    w = arr_i16.reshape(R // 16, 16).T  # [16, R/16]
    return np.tile(w, (8, 1))


def _wrap128(arr):
    """[R] -> [128, R//128] (edge j at [j%128, j//128])."""
    R = arr.shape[0]
    return np.ascontiguousarray(arr.reshape(R // 128, 128).T)


def _host_prep(inputs):
    """Returns (meta, per_core_inputs). meta is SPMD-identical; arrays differ per core."""
    xA = np.asarray(inputs["xA"], np.float32)
    xB = np.asarray(inputs["xB"], np.float32)
    Wk = np.asarray(inputs["Wk"], np.float32)
    bk = np.asarray(inputs["bk"], np.float32)
    Wq = np.asarray(inputs["Wq"], np.float32)
    bq = np.asarray(inputs["bq"], np.float32)
    Wv = np.asarray(inputs["Wv"], np.float32)
    bv = np.asarray(inputs["bv"], np.float32)
    Wa = np.asarray(inputs["Wa"], np.float32)
    ba = np.asarray(inputs["ba"], np.float32)
    skip = np.asarray(inputs["skip"], np.float32)
    a_rel = np.asarray(inputs["a_rel"], np.float32)
    m_rel = np.asarray(inputs["m_rel"], np.float32)
    p_rel = np.asarray(inputs["p_rel"], np.float32)

    beta = _sigmoid(skip)  # [2]

    # ---- fused weights ----
    # A-type src table: [ vr(rel2) | kA | vr(rel0) ] so each relation's k|v
    # pair is one contiguous 256-element gather row: rel0 reads cols 128:384
    # as [k|v], rel2 reads cols 0:256 as [v|k].
    blkM = [_blockdiag(m_rel[r]) for r in range(3)]
    wfa = np.concatenate([Wv[0] @ blkM[2], Wk[0], Wv[0] @ blkM[0]], axis=1)  # [128,384]
    bfa = np.concatenate([bv[0] @ blkM[2], bk[0], bv[0] @ blkM[0]])          # [384]
    wfb = np.concatenate([Wk[1], Wv[1] @ blkM[1]], axis=1)                   # [128,256]
    bfb = np.concatenate([bk[1], bv[1] @ blkM[1]])
    # qr weights: qr_r = q_t(r) @ blkdiag(a_rel[r].T) * scale * p_rel[r,h]
    blkQ = []
    for r in range(3):
        mats = [a_rel[r, h].T * (SCALE * p_rel[r, h]) for h in range(H)]
        blkQ.append(_blockdiag(np.stack(mats)))
    # dst types: rel0 -> B, rel1 -> A, rel2 -> A
    wqb = Wq[1] @ blkQ[0]
    bqb = bq[1] @ blkQ[0]
    wqa = np.concatenate([Wq[0] @ blkQ[1], Wq[0] @ blkQ[2]], axis=1)  # [128,256]
    bqa = np.concatenate([bq[0] @ blkQ[1], bq[0] @ blkQ[2]])

    # ---- consts ----
    iota = np.tile(np.arange(SCN + 30, dtype=np.float32)[None, :NCH * 128], (P, 1)).astype(np.float16)
    blkd = np.zeros((C, H), np.float16)
    for h in range(H):
        blkd[h * DH:(h + 1) * DH, h] = 1.0
    ones1 = np.ones((1, C), np.float16)
    ident = np.eye(P, dtype=np.float16)

    # ---- per-type padded fp16 x ----
    def pad_rows(a, n):
        out = np.zeros((n, a.shape[1]), a.dtype)
        out[: a.shape[0]] = a
        return out

    xA16 = np.ascontiguousarray(pad_rows(xA.astype(np.float16), NPAD).T)  # [C, NPAD]
    xB16 = np.ascontiguousarray(pad_rows(xB.astype(np.float16), NPAD).T)

    # ---- edge prep ----
    meta = {"Rt": [], "visits": [], "tot16": [], "tot128": []}
    per_core = [dict() for _ in range(M)]
    rng_extra = 0
    for ri, (ekey, styp, dtyp) in enumerate(RELS):
        e = np.asarray(inputs[ekey])
        src = e[0].astype(np.int64)
        dst = e[1].astype(np.int64)
        core = dst // NT
        scid = (dst % NT) // SCN
        sub = src // SUBN
        key = (core * NSC + scid) * SUBT + sub
        order = np.lexsort((dst, key))
        src_s = src[order]
        dst_s = dst[order]
        key_s = key[order]
        counts = np.bincount(key_s, minlength=M * NSC * SUBT).reshape(M, NSC, SUBT)
        Rt = np.maximum(128, ((counts.max(axis=0) + 127) // 128) * 128)  # [NSC, SUBT]
        starts = np.zeros(M * NSC * SUBT + 1, np.int64)
        np.cumsum(counts.reshape(-1), out=starts[1:])

        # per-core arrays + per-batch chunk spans
        tot16 = int(Rt.sum() // 16)
        tot128 = int(Rt.sum() // 128)
        spans = {}  # (sc, sub, b) -> [cmin, cmax] union over cores
        for m in range(M):
            ik = np.zeros(int(Rt.sum()), np.int16)
            iq = np.zeros(int(Rt.sum()), np.int16)
            dr = np.full(int(Rt.sum()), -1.0, np.float16)
            off = 0
            for sc in range(NSC):
                for su in range(SUBT):
                    R = int(Rt[sc, su])
                    k = (m * NSC + sc) * SUBT + su
                    lo, hi = int(starts[k]), int(starts[k + 1])
                    n = hi - lo
                    ik[off:off + n] = (src_s[lo:hi] - su * SUBN).astype(np.int16)
                    iq[off:off + n] = (dst_s[lo:hi] - m * NT).astype(np.int16)
                    dl = (dst_s[lo:hi] - m * NT - sc * SCN).astype(np.int32)
                    dr[off:off + n] = dl.astype(np.float16)
                    for b in range(R // 128):
                        if b * 128 >= n:
                            break
                        c0 = int(dl[b * 128]) // 128
                        c1 = int(dl[min(b * 128 + 127, n - 1)]) // 128
                        kk = (sc, su, b)
                        if kk in spans:
                            spans[kk][0] = min(spans[kk][0], c0)
                            spans[kk][1] = max(spans[kk][1], c1)
                        else:
                            spans[kk] = [c0, c1]
                    off += R
            per_core[m][f"idxk{ri}"] = _wrap16(ik)
            per_core[m][f"idxq{ri}"] = _wrap16(iq)
            per_core[m][f"drel{ri}"] = _wrap128(dr)
            per_core[m][f"drln{ri}"] = _wrap128((-dr).astype(np.float32))

        # build visit lists with per-BANK psum group start/stop flags (PSUM zero
        # regions are 2KB = one bank; only one accumulation group per bank, and
        # start zeroes the whole bank). One fused [msg|e8] matmul per visit:
        # chunk c -> bank c//3, offset (c%3)*136 (128 msg cols + 8 exp-sum cols).
        visits = []  # [sc][sub][b] -> list of (chunk, start, stop)
        for sc in range(NSC):
            order_v = []  # (sub, b, chunk) program order
            for su in range(SUBT):
                for b in range(int(Rt[sc, su]) // 128):
                    sp = spans.get((sc, su, b))
                    if sp is None:
                        continue
                    for c in range(sp[0], sp[1] + 1):
                        order_v.append((su, b, c))
            seen_banks = set(c // 3 for _, _, c in order_v)
            last_su = SUBT - 1
            last_b = int(Rt[sc, last_su]) // 128 - 1
            for bk in range(4):
                if bk not in seen_banks:
                    order_v.append((last_su, last_b, bk * 3))
            first = {}
            last = {}
            for j, (_, _, c) in enumerate(order_v):
                bk = c // 3
                if bk not in first:
                    first[bk] = j
                last[bk] = j
            vl = [[[] for _ in range(int(Rt[sc, su]) // 128)] for su in range(SUBT)]
            for j, (su, b, c) in enumerate(order_v):
                bk = c // 3
                vl[su][b].append((c, j == first[bk], j == last[bk]))
            visits.append(vl)

        meta["Rt"].append([[int(x) for x in row] for row in Rt])
        meta["visits"].append(visits)
        meta["tot16"].append(tot16)
        meta["tot128"].append(tot128)
        rng_extra += int(Rt.sum())

    meta["beta"] = [float(beta[0]), float(beta[1])]
    meta["zfa"] = bool(np.all(bfa == 0))
    meta["zfb"] = bool(np.all(bfb == 0))
    meta["zba"] = [bool(np.all(ba[t] == 0)) for t in (0, 1)]
    meta["zbqa"] = bool(np.all(bqa == 0))
    meta["zbqb"] = bool(np.all(bqb == 0))

    # ---- shared (replicated) inputs ----
    shared = {
        "xA16": xA16, "xB16": xB16,
        "wfa": wfa.astype(np.float16), "bfa": bfa.astype(np.float16)[None, :],
        "wfb": wfb.astype(np.float16), "bfb": bfb.astype(np.float16)[None, :],
        "wqa": wqa.astype(np.float16), "bqa": bqa.astype(np.float16)[None, :],
        "wqb": wqb.astype(np.float16), "bqb": bqb.astype(np.float16)[None, :],
        "waa": Wa[0].astype(np.float16), "baa": ba[0].astype(np.float16)[None, :],
        "wab": Wa[1].astype(np.float16), "bab": ba[1].astype(np.float16)[None, :],
        "iota": iota, "blkd": blkd, "ones1": ones1, "ident": ident,
    }
    for m in range(M):
        r0, r1 = m * NT, (m + 1) * NT
        per_core[m]["xad16"] = np.ascontiguousarray(pad_rows(xA[r0:r1].astype(np.float16), NTPAD).T)
        per_core[m]["xbd16"] = np.ascontiguousarray(pad_rows(xB[r0:r1].astype(np.float16), NTPAD).T)
        per_core[m]["xsa"] = np.ascontiguousarray((1.0 - beta[0]) * xA[r0:r1])
        per_core[m]["xsb"] = np.ascontiguousarray((1.0 - beta[1]) * xB[r0:r1])
        per_core[m].update(shared)
    return meta, per_core


def _build_nc(meta):
    import concourse.bacc as bacc
    import concourse.mybir as mybir
    import concourse.tile as tile

    f16 = mybir.dt.float16
    f32 = mybir.dt.float32
    i16 = mybir.dt.int16
    AF = mybir.ActivationFunctionType
    ALU = mybir.AluOpType

    nc = bacc.Bacc("TRN2", target_bir_lowering=False, debug=False, num_swdge_queues=4)

    # ---- I/O ----
    def din(name, shape, dt):
        return nc.dram_tensor(name, shape, dt, kind="ExternalInput")

    xA16 = din("xA16", [C, NPAD], f16)
    xB16 = din("xB16", [C, NPAD], f16)
    xad16 = din("xad16", [C, NTPAD], f16)
    xbd16 = din("xbd16", [C, NTPAD], f16)
    xsa = din("xsa", [NT, C], f32)
    xsb = din("xsb", [NT, C], f32)
    wfa = din("wfa", [C, 384], f16)
    bfa = din("bfa", [1, 384], f16)
    wfb = din("wfb", [C, 256], f16)
    bfb = din("bfb", [1, 256], f16)
    wqa = din("wqa", [C, 256], f16)
    bqa = din("bqa", [1, 256], f16)
    wqb = din("wqb", [C, 128], f16)
    bqb = din("bqb", [1, 128], f16)
    waa = din("waa", [C, C], f16)
    baa = din("baa", [1, C], f16)
    wab = din("wab", [C, C], f16)
    bab = din("bab", [1, C], f16)
    iota_d = din("iota", [P, NCH * 128], f16)
    blkd_d = din("blkd", [C, H], f16)
    ones1_d = din("ones1", [1, C], f16)
    ident_d = din("ident", [P, P], f16)
    idx_d = []
    for r in range(3):
        idx_d.append((
            din(f"idxk{r}", [P, meta["tot16"][r]], i16),
            din(f"idxq{r}", [P, meta["tot16"][r]], i16),
            din(f"drel{r}", [P, meta["tot128"][r]], f16),
            din(f"drln{r}", [P, meta["tot128"][r]], f32),
        ))
    outA = nc.dram_tensor("outA", [NT, C], f32, kind="ExternalOutput")
    outB = nc.dram_tensor("outB", [NT, C], f32, kind="ExternalOutput")

    Rt = meta["Rt"]
    visits = meta["visits"]
    betaA, betaB = meta["beta"]

    with tile.TileContext(nc) as tc:
        with tc.tile_pool(name="dram", bufs=1, space="DRAM") as dram:
            fusedA = dram.tile([NPAD, 384], f16)
            fusedB = dram.tile([NPAD, 256], f16)
            qra = dram.tile([NTPAD, 256], f16)
            qrb = dram.tile([NTPAD, 128], f16)

            with tc.tile_pool(name="const", bufs=1) as cp:
                iota_sb = cp.tile([P, NCH * 128], f16)
                nc.sync.dma_start(iota_sb[:], iota_d[:])
                blkd_sb = cp.tile([C, H], f16)
                nc.sync.dma_start(blkd_sb[:], blkd_d[:])
                ones1_sb = cp.tile([1, C], f16)
                nc.sync.dma_start(ones1_sb[:], ones1_d[:])
                ident_sb = cp.tile([P, P], f16)
                nc.sync.dma_start(ident_sb[:], ident_d[:])
                w_sb = {}
                src_map = {"wfa": wfa, "bfa": bfa, "wfb": wfb, "bfb": bfb,
                           "wqa": wqa, "bqa": bqa, "wqb": wqb, "bqb": bqb,
                           "waa": waa, "baa": baa, "wab": wab, "bab": bab}
                for nm, dt_, sh in [("wfa", f16, [C, 384]), ("bfa", f16, [1, 384]),
                                    ("wfb", f16, [C, 256]), ("bfb", f16, [1, 256]),
                                    ("wqa", f16, [C, 256]), ("bqa", f16, [1, 256]),
                                    ("wqb", f16, [C, 128]), ("bqb", f16, [1, 128]),
                                    ("waa", f16, [C, C]), ("baa", f16, [1, C]),
                                    ("wab", f16, [C, C]), ("bab", f16, [1, C])]:
                    t = cp.tile(sh, dt_, tag=nm)
                    nc.sync.dma_start(t[:], src_map[nm][:])
                    w_sb[nm] = t

                import os as _osr
                _REP = int(_osr.environ.get("KERNEL_REPEAT", "1"))
                for _rep in range(_REP):
                    # ================= PHASE 1: build tables =================
                    with tc.tile_pool(name="prep", bufs=4) as pp, \
                         tc.tile_pool(name="prep_ps", bufs=3, space="PSUM") as pps:

                        def build_table(xdram, nrows, w, b, tbl, ncols, skip_bias):
                            GRP = 4  # node-tiles per DMA batch
                            ntiles = nrows // 128
                            base = 0
                            ii = 0
                            while base < ntiles:
                                grp = min(GRP, ntiles - base)
                                r0 = base * 128
                                xT = pp.tile([P, GRP, P], f16, tag="xT")
                                nc.sync.dma_start(
                                    xT[:, :grp, :], xdram[:, r0:r0 + grp * 128]
                                    .rearrange("c (t n) -> c t n", t=grp))
                                sb = pp.tile([P, GRP, ncols], f16, tag=f"sb{ncols}")
                                for t in range(grp):
                                    ps = pps.tile([P, 512], f32, tag="ps")
                                    nc.tensor.matmul(ps[:, :ncols], xT[:, t, :], w[:],
                                                     start=True, stop=skip_bias)
                                    if not skip_bias:
                                        nc.tensor.matmul(ps[:, :ncols], ones1_sb[:], b[:],
                                                         start=False, stop=True)
                                    nc.scalar.copy(sb[:, t, :], ps[:, :ncols])
                                    ii += 1
                                nc.scalar.dma_start(
                                    tbl[r0:r0 + grp * 128, :]
                                    .rearrange("(t n) c -> n t c", n=128), sb[:, :grp, :])
                                base += grp

                        import os as _os1
                        if not _os1.environ.get("KERNEL_NOPREP"):
                            build_table(xA16, NPAD, w_sb["wfa"], w_sb["bfa"], fusedA, 384,
                                        meta["zfa"])
                            build_table(xB16, NPAD, w_sb["wfb"], w_sb["bfb"], fusedB, 256,
                                        meta["zfb"])
                            build_table(xad16, NTPAD, w_sb["wqa"], w_sb["bqa"], qra, 256,
                                        meta["zbqa"])
                            build_table(xbd16, NTPAD, w_sb["wqb"], w_sb["bqb"], qrb, 128,
                                        meta["zbqb"])

                    # ================= PHASE 2: streaming =================
                    with tc.tile_pool(name="agg", bufs=1) as apool:
                        agg = apool.tile([P, NSC * NCH, C], f16)

                        def out_stage(t):
                            import os as _os2
                            if _os2.environ.get("KERNEL_NOOUT"):
                                return
                            xs_d = xsa if t == 0 else xsb
                            out_d = outA if t == 0 else outB
                            wa = w_sb["waa" if t == 0 else "wab"]
                            bb = w_sb["baa" if t == 0 else "bab"]
                            bt = betaA if t == 0 else betaB
                            with tc.tile_pool(name="op", bufs=4) as op, \
                                 tc.tile_pool(name="ops", bufs=2, space="PSUM") as ops:
                                for slot in range(NSC * NCH):
                                    sc, ch = divmod(slot, NCH)
                                    rows = 98 if ch == 9 else 128
                                    base = sc * SCN + ch * 128
                                    g16 = op.tile([P, C], f16, tag="g16")
                                    nc.scalar.activation(g16[:], agg[:, slot, :], AF.Gelu)
                                    gt = ops.tile([P, C], f16, tag="gt")
                                    nc.tensor.transpose(gt[:], g16[:], ident_sb[:])
                                    gts = op.tile([P, C], f16, tag="gts")
                                    nc.vector.tensor_copy(gts[:], gt[:])
                                    o_ps = ops.tile([P, C], f32, tag="o")
                                    zb = meta["zba"][t]
                                    nc.tensor.matmul(o_ps[:], gts[:], wa[:], start=True, stop=zb)
                                    if not zb:
                                        nc.tensor.matmul(o_ps[:], ones1_sb[:], bb[:],
                                                         start=False, stop=True)
                                    xs = op.tile([P, C], f32, tag="xs")
                                    nc.sync.dma_start(xs[:rows, :], xs_d[base:base + rows, :])
                                    ob = op.tile([P, C], f32, tag="ob")
                                    nc.scalar.activation(ob[:], o_ps[:], AF.Copy, scale=float(bt))
                                    res = op.tile([P, C], f32, tag="res")
                                    nc.vector.tensor_add(res[:rows, :], ob[:rows, :], xs[:rows, :])
                                    nc.sync.dma_start(out_d[base:base + rows, :], res[:rows, :])

                        with tc.tile_pool(name="gidx", bufs=1) as gi, \
                             tc.tile_pool(name="gkv", bufs=4) as gkv, \
                             tc.tile_pool(name="gp", bufs=3) as gp, \
                             tc.tile_pool(name="ep", bufs=4) as ep:
                            import os as _os
                            n_rel = int(_os.environ.get("KERNEL_NREL", "3"))
                            obc = [0]  # one-hot engine split counter
                            gq = [0]  # round-robin swdge queue cursor

                            def nxq():
                                gq[0] = (gq[0] + 1) % 4
                                return gq[0]

                            for r, (ekey, styp, dtyp) in enumerate(RELS[:n_rel]):
                                ftab, fw = (fusedA, 384) if styp == 0 else (fusedB, 256)
                                # rel0: cols 128:384 = [k|v]; rel2: cols 0:256 = [v|k]; rel1: [k|v]
                                kvcol = 0 if r == 2 else (128 if r == 0 else 0)
                                ks, vs = ((128, 0) if r == 2 else (0, 128))
                                if r == 0:
                                    qtab, qw, qoff = qrb, 128, 0
                                elif r == 1:
                                    qtab, qw, qoff = qra, 256, 0
                                else:
                                    qtab, qw, qoff = qra, 256, 128
                                qap = qtab[:, qoff:qoff + 128]

                                idxk_sb = gi.tile([P, meta["tot16"][r]], i16, tag="idxk")
                                nc.sync.dma_start(idxk_sb[:], idx_d[r][0][:])
                                idxq_sb = gi.tile([P, meta["tot16"][r]], i16, tag="idxq")
                                nc.sync.dma_start(idxq_sb[:], idx_d[r][1][:])
                                drel_sb = gi.tile([P, meta["tot128"][r]], f16, tag="drel")
                                nc.sync.dma_start(drel_sb[:], idx_d[r][2][:])
                                drln_sb = gi.tile([P, meta["tot128"][r]], f32, tag="drln")
                                nc.sync.dma_start(drln_sb[:], idx_d[r][3][:])

                                with tc.tile_pool(name=f"agps{r}", bufs=2, space="PSUM") as agps:
                                    off16 = 0
                                    off128 = 0
                                    for sc in range(NSC):
                                        ag = agps.tile([P, 4, 512], f32, tag="aggps")
                                        for su in range(SUBT):
                                            R = Rt[r][sc][su]
                                            B = R // 128
                                            kvap = ftab[su * SUBN:(su + 1) * SUBN, kvcol:kvcol + 256]
                                            GC = 896  # per-gather idx cap (desc carveout is 1024)
                                            kv = gkv.tile([P, B, 256], f16, tag="kv")
                                            qg = gkv.tile([P, B, 128], f16, tag="qg")
                                            for j0 in range(0, R, GC):
                                                n = min(GC, R - j0)
                                                i16s = idxk_sb[:, off16 + j0 // 16:off16 + (j0 + n) // 16]
                                                i16q = idxq_sb[:, off16 + j0 // 16:off16 + (j0 + n) // 16]
                                                nc.gpsimd.dma_gather(
                                                    kv[:, j0 // 128:(j0 + n) // 128, :], kvap, i16s,
                                                    n, n, 256, elem_step=fw, transpose=False,
                                                    queue_num=nxq())
                                                nc.gpsimd.dma_gather(
                                                    qg[:, j0 // 128:(j0 + n) // 128, :], qap, i16q,
                                                    n, n, 128, elem_step=qw, transpose=False,
                                                    queue_num=nxq())
                                            prod = gp.tile([P, B, 128], f16, tag="prod")
                                            if not _os.environ.get("KERNEL_NOPROD"):
                                                nc.vector.tensor_mul(prod[:], kv[:, :, ks:ks + 128], qg[:])
                                            lpr = gp.tile([P, B, H], f16, tag="lp")
                                            with nc.allow_low_precision("16-term head reduce; logits O(6)"):
                                                nc.vector.tensor_reduce(
                                                    lpr[:], prod[:].rearrange("p b (h d) -> p b h d", d=DH),
                                                    mybir.AxisListType.X, ALU.add)
                                            msg = gp.tile([P, B, 136], f16, tag="msg")
                                            nc.scalar.activation(msg[:, :, 128:136], lpr[:], AF.Exp)
                                            if not _os.environ.get("KERNEL_NOMSGMUL"):
                                                nc.vector.tensor_tensor(
                                                    out=msg[:, :, 0:128]
                                                    .rearrange("p b (h d) -> p b h d", d=DH),
                                                    in0=kv[:, :, vs:vs + 128]
                                                    .rearrange("p b (h d) -> p b h d", d=DH),
                                                    in1=msg[:, :, 128:136]
                                                    .to_broadcast([P, B, H, DH]),
                                                    op=ALU.mult)
                                            for b in range(B):
                                                vlist = visits[r][sc][su][b]
                                                if vlist and not _os.environ.get("KERNEL_NOONEHOT"):
                                                    c0 = min(ch for ch, _, _ in vlist)
                                                    c1 = max(ch for ch, _, _ in vlist)
                                                    w0, w1 = c0 * 128, (c1 + 1) * 128
                                                    ohw = gp.tile([P, NCH * 128], f16, tag="ohw")
                                                    obc[0] += 1
                                                    if obc[0] % 4 == 3:
                                                        oht = gp.tile([P, NCH * 128], f16, tag="oht")
                                                        nc.scalar.activation(
                                                            oht[:, w0:w1], iota_sb[:, w0:w1], AF.Abs,
                                                            bias=drln_sb[:, off128 + b:off128 + b + 1])
                                                        nc.scalar.activation(
                                                            ohw[:, w0:w1], oht[:, w0:w1], AF.Relu,
                                                            bias=1.0, scale=-1.0)
                                                    else:
                                                        nc.vector.tensor_tensor(
                                                            out=ohw[:, w0:w1],
                                                            in0=iota_sb[:, w0:w1],
                                                            in1=drel_sb[:, off128 + b:off128 + b + 1]
                                                            .to_broadcast([P, w1 - w0]),
                                                            op=ALU.is_equal)
                                                for (ch, ast, asp) in vlist:
                                                    oh_ap = (ohw[:, ch * 128:(ch + 1) * 128]
                                                             if not _os.environ.get("KERNEL_NOONEHOT")
                                                             else iota_sb[:, ch * 128:(ch + 1) * 128])
                                                    bk_, col = divmod(ch, 3)
                                                    nc.tensor.matmul(
                                                        ag[:, bk_, col * 136:col * 136 + 136],
                                                        oh_ap, msg[:, b, :], start=ast, stop=asp)
                                            off16 += R // 16
                                            off128 += B
                                        # epilogue for this superchunk
                                        for ch in range(NCH):
                                            bk_, col = divmod(ch, 3)
                                            a_ap = ag[:, bk_, col * 136:col * 136 + 128]
                                            s_ap = ag[:, bk_, col * 136 + 128:col * 136 + 136]
                                            rec = ep.tile([P, H], f32, tag="rec")
                                            nc.vector.tensor_scalar(rec[:], s_ap, 1e-16, None, op0=ALU.add)
                                            rec2 = ep.tile([P, H], f32, tag="rec2")
                                            nc.vector.reciprocal(rec2[:], rec[:])
                                            slot = sc * NCH + ch
                                            tgt = agg[:, slot, :].rearrange("p (h d) -> p h d", d=DH)
                                            src_v = a_ap.rearrange("p (h d) -> p h d", d=DH)
                                            if r == 2:
                                                tmp = ep.tile([P, C], f16, tag="tmp")
                                                nc.vector.tensor_tensor(
                                                    out=tmp[:].rearrange("p (h d) -> p h d", d=DH),
                                                    in0=src_v, in1=rec2[:].to_broadcast([P, H, DH]),
                                                    op=ALU.mult)
                                                nc.vector.tensor_add(agg[:, slot, :], agg[:, slot, :], tmp[:])
                                            else:
                                                nc.vector.tensor_tensor(
                                                    out=tgt, in0=src_v,
                                                    in1=rec2[:].to_broadcast([P, H, DH]),
                                                    op=ALU.mult)
                                if r == 0:
                                    out_stage(1)
                            out_stage(0)
    nc.compile()
    return nc


def _meta_key(meta):
    import json
    return json.dumps(meta, sort_keys=True)


def kernel(**inputs):
    meta, per_core = _host_prep(inputs)
    key = _meta_key(meta)
    if key not in _CACHE:
        _CACHE.clear()
        _CACHE[key] = _build_nc(meta)
    nc = _CACHE[key]

    from concourse.bass_utils import run_bass_kernel_spmd
    import os
    trace = bool(int(os.environ.get("KERNEL_TRACE", "0")))
    res = run_bass_kernel_spmd(nc, per_core, core_ids=list(range(M)), trace=trace)
    if trace:
        kernel.last_exec_time_ns = res.exec_time_ns
        kernel.last_trace = res.instructions_and_trace
    outs = res.results
    outA = np.concatenate([outs[m]["outA"] for m in range(M)], axis=0)
    outB = np.concatenate([outs[m]["outB"] for m in range(M)], axis=0)
    return np.stack([outA, outB]).astype(np.float32)

